# revision 1
# baseline (speedup 1.0000x reference)
"""Trainium2 Bass kernel for nn_CosSimRouter_learn_49778670960796.

Host: cosine-similarity scoring / sort / gather (tiny, shape-determining).
Device (8 NeuronCores, tensor-parallel over heads/hidden):
  3x MHA + FFN + logits; fp32 storage, float32r matmuls; AllReduce after
  out-proj / FFN2 (Megatron-style TP). Activations feature-major [E, L].
Host: top-k + final gather (exact rows of the input).
"""

import numpy as np

E = 4096
H = 16
HID = 8192
GAMMA = 0.2
TEMP = 0.05
EXPAND = 0.7
NCORES = 8
ET = E // 128  # 32 feature tiles
DH = E // H  # 256
HL = H // NCORES  # 2 heads per core
DLOC = HL * DH  # 512 local head dims
FLOC = HID // NCORES  # 1024 local ffn hidden

_CACHE = {}


# ----------------------------------------------------------------------------
# host-side reference math (numpy, fp32) for the scoring stage + fallback
# ----------------------------------------------------------------------------

def _score_partition(vision_feature, text_embed, attention_mask):
    vf = vision_feature.astype(np.float32)
    te = text_embed.astype(np.float32)
    vn = vf / np.maximum(np.linalg.norm(vf, axis=-1, keepdims=True), 1e-8)
    tn = te / np.maximum(np.linalg.norm(te, axis=-1, keepdims=True), 1e-8)
    cs = vn @ tn.T
    cs = np.where(attention_mask[None, :], cs, np.float32(0.0))
    m = cs.max(axis=-1) / np.float32(TEMP)
    e = np.exp(m - m.max())
    scores = e / e.sum()
    order = np.argsort(-scores, kind="stable")
    cum = np.cumsum(scores[order])
    t = int((cum <= GAMMA).sum())
    return t, order[:t], order[t:]


def _ln_np(x):
    m = x.mean(-1, keepdims=True)
    v = ((x - m) ** 2).mean(-1, keepdims=True)
    return (x - m) / np.sqrt(v + 1e-5)


def _gelu_np(x):
    import math

    erf = np.frompyfunc(math.erf, 1, 1)
    return (x * 0.5 * (1.0 + erf(x / math.sqrt(2.0)).astype(np.float64))
            ).astype(x.dtype)


def _mha_np(q_in, kv_in, Wqkv, bqkv, Wo, bo):
    dh = E // H
    Wq, Wk, Wv = np.split(Wqkv, 3, axis=0)
    bq, bk, bv = np.split(bqkv, 3)
    q = (q_in @ Wq.T + bq).reshape(-1, H, dh)
    k = (kv_in @ Wk.T + bk).reshape(-1, H, dh)
    v = (kv_in @ Wv.T + bv).reshape(-1, H, dh)
    att = np.einsum("qhd,khd->hqk", q, k) / np.float32(np.sqrt(dh))
    att = att - att.max(-1, keepdims=True)
    att = np.exp(att)
    att /= att.sum(-1, keepdims=True)
    o = np.einsum("hqk,khd->qhd", att.astype(np.float32), v).reshape(-1, E)
    return o @ Wo.T + bo


def _reference_np(vision_feature, text_embed, attention_mask,
                  Wqkv1, bqkv1, Wo1, bo1, Wqkv2, bqkv2, Wo2, bo2,
                  Wqkvc, bqkvc, Woc, boc, Wf1, bf1, Wf2, bf2, Ws, bs):
    t, sel_idx, rem_idx = _score_partition(vision_feature, text_embed,
                                           attention_mask)
    sel = vision_feature[sel_idx]
    rem = vision_feature[rem_idx]
    cat = np.concatenate([sel, text_embed], axis=0)
    x = _ln_np(_mha_np(cat, cat, Wqkv1, bqkv1, Wo1, bo1) + cat)
    r = _ln_np(_mha_np(rem, rem, Wqkv2, bqkv2, Wo2, bo2) + rem)
    x = _ln_np(_mha_np(r, x, Wqkvc, bqkvc, Woc, boc) + r)
    ffn = _gelu_np(x @ Wf1.T + bf1) @ Wf2.T + bf2
    x = _ln_np(x + ffn)
    logits = (x @ Ws.T + bs).squeeze(-1)
    es = 1.0 / (1.0 + np.exp(-logits))
    k = int(t * EXPAND)
    ei = np.argsort(-es, kind="stable")[:k]
    final = np.sort(np.concatenate([sel_idx, rem_idx[ei]]))
    return vision_feature[final]


# ----------------------------------------------------------------------------
# device program
# ----------------------------------------------------------------------------

def _pad128(n):
    return ((n + 127) // 128) * 128


def _build_device(ncat_real, nrem_real, debug=False):
    import concourse.bacc as bacc
    import concourse.mybir as mybir
    import concourse.tile as tile

    dt = mybir.dt
    F32 = dt.float32
    F32R = dt.float32r
    BF16 = dt.bfloat16
    AF = mybir.ActivationFunctionType
    ALU = mybir.AluOpType

    ncat = _pad128(ncat_real)
    nrem = _pad128(nrem_real)
    JC = ncat // 128  # kv tiles for cat (2)
    JR = nrem // 128  # kv tiles for rem (4)

    nc = bacc.Bacc("TRN2", target_bir_lowering=False, debug=False,
                   num_devices=NCORES)

    # ---------------- DRAM I/O ----------------
    catT_d = nc.dram_tensor("catT", [E, ncat], F32R, kind="ExternalInput")
    remT_d = nc.dram_tensor("remT", [E, nrem], F32R, kind="ExternalInput")
    wd = {}
    for l in ("1", "2", "c"):
        for p in ("q", "k", "v"):
            wd[p + l] = nc.dram_tensor(f"w{p}{l}", [E, DLOC], F32R,
                                       kind="ExternalInput")
        wd["o" + l] = nc.dram_tensor(f"wo{l}", [DLOC, E], F32R,
                                     kind="ExternalInput")
    wd["f1"] = nc.dram_tensor("wf1", [E, FLOC], F32R, kind="ExternalInput")
    wd["f2"] = nc.dram_tensor("wf2", [FLOC, E], F32R, kind="ExternalInput")
    wd["s"] = nc.dram_tensor("ws", [128, ET], F32R, kind="ExternalInput")
    wsb_d = nc.dram_tensor("wsb", [128, E // NCORES // 128], F32R,
                           kind="ExternalInput")
    masks_d = nc.dram_tensor("masks", [128, 4], F32R, kind="ExternalInput")
    logits_d = nc.dram_tensor("logits", [1, nrem], F32, kind="ExternalOutput")
    dbg = {}
    if debug:
        for nm, L in (("dbg_x1", ncat), ("dbg_r", nrem), ("dbg_x2", nrem),
                      ("dbg_x3", nrem)):
            dbg[nm] = nc.dram_tensor(nm, [E, L], F32, kind="ExternalOutput")

    replica = [list(range(NCORES))]

    with tile.TileContext(nc, num_cores=NCORES) as tc:
        with (
            tc.tile_pool(name="acts", bufs=1) as acts,
            tc.tile_pool(name="psum", bufs=1, space="PSUM") as psum,
            tc.tile_pool(name="dram", bufs=1, space="DRAM") as dram,
        ):
            # ---- constants / packed stat tiles ----
            ones_col = acts.tile([128, 1], F32R, name="ones_col",
                                 tag="ones_col")
            nc.vector.memset(ones_col[:].bitcast(F32), 1.0)
            ones_row = acts.tile([1, 128], F32R, name="ones_row",
                                 tag="ones_row")
            nc.vector.memset(ones_row[:].bitcast(F32), 1.0)
            masks = acts.tile([128, 4], F32R, name="masks", tag="masks")
            nc.sync.dma_start(masks[:], masks_d.ap())

            def pp(name, L):
                return psum.tile([128, L], F32, name=name, tag="pp", bufs=8)

            def pstat(name, L):
                return psum.tile([1, L], F32, name=name, tag="pp", bufs=8)

            def wtile(name, cols):
                return acts.tile([128, cols], F32R, name=name, tag="wt",
                                 bufs=6, padded_shape=[128, 1024])

            # ---------------- building blocks ----------------
            def load_xT(name, dram_t, L, tagbase):
                ts = []
                for k in range(ET):
                    xt = acts.tile([128, L], F32R, name=f"{name}_{k}",
                                   tag=f"{tagbase}_{k}")
                    nc.sync.dma_start(xt[:],
                                      dram_t.ap()[128 * k:128 * (k + 1), :])
                    ts.append(xt)
                return ts

            def proj_fm(tagbase, w_dram, x_tiles, L, outtag):
                """q/k fm projection -> 4 tiles [128, L] (f32r)."""
                ps = [pp(f"ps_{tagbase}_{m}", L) for m in range(4)]
                outs = []
                for k in range(ET):
                    wt = wtile(f"w_{tagbase}_{k}", DLOC)
                    nc.sync.dma_start(
                        wt[:], w_dram.ap()[128 * k:128 * (k + 1), :])
                    for m in range(4):
                        nc.tensor.matmul(ps[m][:],
                                         wt[:, 128 * m:128 * (m + 1)],
                                         x_tiles[k][:],
                                         start=(k == 0), stop=(k == ET - 1))
                for m in range(4):
                    o = acts.tile([128, L], F32R, name=f"{tagbase}_{m}",
                                  tag=f"{outtag}_{m}")
                    nc.vector.tensor_copy(o[:], ps[m][:])
                    outs.append(o)
                return outs

            def proj_tm(tagbase, w_dram, x_tiles, L):
                """v tm projection -> L//128 tiles [128, DLOC] (f32r)."""
                jt = L // 128
                ps = [pp(f"ps_{tagbase}_{j}", DLOC) for j in range(jt)]
                outs = []
                for k in range(ET):
                    wt = wtile(f"w_{tagbase}_{k}", DLOC)
                    nc.sync.dma_start(
                        wt[:], w_dram.ap()[128 * k:128 * (k + 1), :])
                    for j in range(jt):
                        nc.tensor.matmul(ps[j][:],
                                         x_tiles[k][:, 128 * j:128 * (j + 1)],
                                         wt[:],
                                         start=(k == 0), stop=(k == ET - 1))
                for j in range(jt):
                    o = acts.tile([128, DLOC], F32R, name=f"{tagbase}_{j}",
                                  tag=f"v_{j}")
                    nc.vector.tensor_copy(o[:], ps[j][:])
                    outs.append(o)
                return outs

            def attention(tag, qT, kT, vT, Lq, Lkv, kv_valid, mask_idx):
                jt = Lkv // 128
                oT = []
                for h in range(HL):
                    exps = []
                    for j in range(jt):
                        p = pp(f"ps_s_{tag}_{h}_{j}", Lq)
                        for c in range(2):
                            nc.tensor.matmul(
                                p[:],
                                kT[2 * h + c][:, 128 * j:128 * (j + 1)],
                                qT[2 * h + c][:],
                                start=(c == 0), stop=(c == 1))
                        e = acts.tile([128, Lq], F32R,
                                      name=f"es_{tag}_{h}_{j}",
                                      tag=f"expS_{j}")
                        nc.scalar.activation(e[:], p[:], AF.Exp,
                                             scale=float(1.0 / np.sqrt(DH)))
                        exps.append(e)
                    dsum = pstat(f"ps_d_{tag}_{h}", Lq)
                    for j in range(jt):
                        if j == jt - 1 and kv_valid < Lkv:
                            col = masks[:, mask_idx:mask_idx + 1]
                        else:
                            col = ones_col[:]
                        nc.tensor.matmul(dsum[:], col, exps[j][:],
                                         start=(j == 0), stop=(j == jt - 1))
                    den = acts.tile([1, Lq], F32, name=f"den_{tag}_{h}",
                                    tag="aden")
                    rec = acts.tile([1, Lq], F32, name=f"rec_{tag}_{h}",
                                    tag="arec")
                    nc.vector.tensor_copy(den[:], dsum[:])
                    nc.vector.reciprocal(rec[:], den[:])
                    nc.vector.tensor_tensor(den[:], den[:], rec[:], ALU.mult)
                    nc.vector.tensor_scalar(den[:], den[:], -1.0, 2.0,
                                            ALU.mult, ALU.add)
                    rec2 = acts.tile([1, Lq], F32R, name=f"rec2_{tag}_{h}",
                                     tag="rec2")
                    nc.vector.tensor_tensor(rec2[:], rec[:], den[:], ALU.mult)
                    rrep_p = pp(f"ps_rr_{tag}_{h}", Lq)
                    nc.tensor.matmul(rrep_p[:], ones_row[:], rec2[:],
                                     start=True, stop=True)
                    rrep = acts.tile([128, Lq], F32, name=f"rr_{tag}_{h}",
                                     tag="rrep")
                    nc.scalar.copy(rrep[:], rrep_p[:])
                    for c in range(2):
                        po = pp(f"ps_o_{tag}_{h}_{c}", Lq)
                        for j in range(jt):
                            nc.tensor.matmul(
                                po[:],
                                vT[j][:, 256 * h + 128 * c:
                                      256 * h + 128 * (c + 1)],
                                exps[j][:],
                                start=(j == 0), stop=(j == jt - 1))
                        o = acts.tile([128, Lq], F32R,
                                      name=f"oT_{tag}_{h}_{c}",
                                      tag=f"oT_{2 * h + c}")
                        nc.vector.tensor_tensor(o[:], po[:], rrep[:],
                                                ALU.mult)
                        oT.append(o)
                return oT

            def out_proj_to_dram(tag, oT, w_dram, ar_in, Lq, sdt):
                for quarter in range(4):
                    wo_t = []
                    for k in range(4):
                        wt = wtile(f"wo_{tag}_{quarter}_{k}", 1024)
                        nc.sync.dma_start(
                            wt[:],
                            w_dram.ap()[128 * k:128 * (k + 1),
                                        1024 * quarter:1024 * (quarter + 1)])
                        wo_t.append(wt)
                    ps = []
                    for mm in range(8):
                        m = 8 * quarter + mm
                        ps.append(pp(f"ps_op_{tag}_{m}", Lq))
                    for k in range(4):
                        for mm in range(8):
                            nc.tensor.matmul(
                                ps[mm][:],
                                wo_t[k][:, 128 * mm:128 * (mm + 1)],
                                oT[k][:],
                                start=(k == 0), stop=(k == 3))
                    for mm in range(8):
                        m = 8 * quarter + mm
                        st = acts.tile([128, Lq], sdt,
                                       name=f"st_{tag}_{m}", tag="stage",
                                       bufs=3)
                        nc.vector.tensor_copy(st[:], ps[mm][:])
                        nc.sync.dma_start(
                            ar_in[128 * m:128 * (m + 1), :], st[:])

            def do_allreduce(tag, ar_in, ar_out):
                nc.gpsimd.collective_compute(
                    "AllReduce", ALU.add, replica_groups=replica,
                    ins=[ar_in.opt()], outs=[ar_out.opt()])

            def residual_ln(tag, ar_out, res_tiles, L, adt=F32, valid=None,
                            dump=None, normalize=True):
                """In-place: res_tiles[k] <- LN(ar_out + res_tiles)[k]."""
                # xsum (in-place into res slot)
                for k in range(ET):
                    b = acts.tile([128, L], adt, name=f"arb_{tag}_{k}",
                                  tag="arb", bufs=4)
                    nc.sync.dma_start(b[:], ar_out[128 * k:128 * (k + 1), :])
                    nc.vector.tensor_tensor(res_tiles[k][:], b[:],
                                            res_tiles[k][:], ALU.add)
                s1p = pstat(f"ps_s1_{tag}", L)
                s2p = pstat(f"ps_s2_{tag}", L)
                for k in range(ET):
                    nc.tensor.matmul(s1p[:], ones_col[:], res_tiles[k][:],
                                     start=(k == 0), stop=(k == ET - 1))
                for k in range(ET):
                    sq = acts.tile([128, L], F32R, name=f"sq_{tag}_{k}",
                                   tag="stage", bufs=3)
                    nc.scalar.square(sq[:], res_tiles[k][:])
                    nc.tensor.matmul(s2p[:], ones_col[:], sq[:],
                                     start=(k == 0), stop=(k == ET - 1))
                mean = acts.tile([1, L], F32, name=f"mean_{tag}",
                                 tag="lmean")
                var = acts.tile([1, L], F32, name=f"var_{tag}", tag="lvar")
                tmpa = acts.tile([1, L], F32, name=f"tmpa_{tag}", tag="ltmp")
                r0 = acts.tile([1, L], F32, name=f"r0_{tag}", tag="lr0")
                nc.scalar.mul(mean[:], s1p[:], 1.0 / E)
                nc.scalar.mul(var[:], s2p[:], 1.0 / E)
                nc.scalar.square(tmpa[:], mean[:])
                nc.vector.tensor_sub(var[:], var[:], tmpa[:])
                nc.vector.tensor_scalar_add(var[:], var[:], 1e-5)
                nc.scalar.sqrt(tmpa[:], var[:])
                nc.vector.reciprocal(r0[:], tmpa[:])
                nc.vector.tensor_tensor(tmpa[:], r0[:], r0[:], ALU.mult)
                nc.vector.tensor_tensor(tmpa[:], tmpa[:], var[:], ALU.mult)
                nc.vector.tensor_scalar(tmpa[:], tmpa[:], -0.5, 1.5, ALU.mult,
                                        ALU.add)
                rstd = acts.tile([1, L], F32R, name=f"rstd_{tag}", tag="rstd")
                nmr = acts.tile([1, L], F32R, name=f"nmr_{tag}", tag="nmr")
                nc.vector.tensor_tensor(rstd[:], r0[:], tmpa[:], ALU.mult)
                nc.vector.scalar_tensor_tensor(nmr[:], mean[:], -1.0, rstd[:],
                                               ALU.mult, ALU.mult)
                if not normalize:
                    return rstd, nmr
                Apsum = pp(f"ps_A_{tag}", L)
                nc.tensor.matmul(Apsum[:], ones_row[:], rstd[:], start=True,
                                 stop=True)
                Bpsum = pp(f"ps_B_{tag}", L)
                nc.tensor.matmul(Bpsum[:], ones_row[:], nmr[:], start=True,
                                 stop=True)
                Asb = acts.tile([128, L], F32, name=f"A_{tag}", tag="Asb")
                nc.scalar.copy(Asb[:], Apsum[:])
                Bsb = acts.tile([128, L], F32, name=f"B_{tag}", tag="Bsb")
                nc.scalar.copy(Bsb[:], Bpsum[:])
                for k in range(ET):
                    nc.vector.tensor_tensor(res_tiles[k][:], res_tiles[k][:],
                                            Asb[:], ALU.mult)
                    nc.vector.tensor_tensor(res_tiles[k][:], res_tiles[k][:],
                                            Bsb[:], ALU.add)
                    if valid is not None and valid < L:
                        nc.vector.memset(
                            res_tiles[k][:, valid:L].bitcast(F32), 0.0)
                    if dump is not None:
                        nc.sync.dma_start(
                            dump.ap()[128 * k:128 * (k + 1), :],
                            res_tiles[k][:].bitcast(F32))
                return res_tiles

            # ================= program =================
            # fm512 family ("a_{k}"): remT -> r -> x2 -> x3 (in-place chain)
            # fm256 family ("b_{k}"): catT -> x1
            a_t = load_xT("remT", remT_d, nrem, "a")

            # ---- MHA2 (rem self-attention) ----
            q2 = proj_fm("q2", wd["q2"], a_t, nrem, "q")
            k2 = proj_fm("k2", wd["k2"], a_t, nrem, "k")
            v2 = proj_tm("v2", wd["v2"], a_t, nrem)
            o2 = attention("a2", q2, k2, v2, nrem, nrem, nrem_real, 1)
            arin2 = dram.tile([E, nrem], BF16, name="arin2", tag="arin2")
            arout2 = dram.tile([E, nrem], BF16, name="arout2", tag="arout2", addr_space="Shared")
            out_proj_to_dram("op2", o2, wd["o2"], arin2, nrem, BF16)
            do_allreduce("2", arin2, arout2)

            # ---- MHA1 (cat self-attention), overlaps AR2 ----
            b_t = load_xT("catT", catT_d, ncat, "b")
            q1 = proj_fm("q1", wd["q1"], b_t, ncat, "q")
            k1 = proj_fm("k1", wd["k1"], b_t, ncat, "k")
            v1 = proj_tm("v1", wd["v1"], b_t, ncat)
            o1 = attention("a1", q1, k1, v1, ncat, ncat, ncat_real, 0)
            arin1 = dram.tile([E, ncat], BF16, name="arin1", tag="arin1")
            arout1 = dram.tile([E, ncat], BF16, name="arout1", tag="arout1", addr_space="Shared")
            out_proj_to_dram("op1", o1, wd["o1"], arin1, ncat, BF16)
            do_allreduce("1", arin1, arout1)

            # ---- LN stages: r = LN(AR2 + rem); x1 = LN(AR1 + cat) ----
            r_t = residual_ln("r", arout2, a_t, nrem, adt=BF16,
                              dump=dbg.get("dbg_r"))
            x1_t = residual_ln("x1", arout1, b_t, ncat, adt=BF16,
                               valid=ncat_real, dump=dbg.get("dbg_x1"))

            # ---- MHAc (q from r, kv from x1) ----
            qc = proj_fm("qc", wd["qc"], r_t, nrem, "q")
            kc = proj_fm("kc", wd["kc"], x1_t, ncat, "k")
            vc = proj_tm("vc", wd["vc"], x1_t, ncat)
            oc = attention("ac", qc, kc, vc, nrem, ncat, ncat_real, 0)
            arinc = dram.tile([E, nrem], BF16, name="arinc", tag="arinc")
            aroutc = dram.tile([E, nrem], BF16, name="aroutc", tag="aroutc", addr_space="Shared")
            out_proj_to_dram("opc", oc, wd["oc"], arinc, nrem, BF16)
            do_allreduce("c", arinc, aroutc)
            x2_t = residual_ln("x2", aroutc, r_t, nrem, adt=BF16,
                               dump=dbg.get("dbg_x2"))

            # ---- FFN ----
            # f1: hT = gelu(Wf1_shard @ x2): 8 psums, single weight sweep
            ps_f1 = [pp(f"ps_f1_{m}", nrem) for m in range(8)]
            for k in range(ET):
                wt = wtile(f"w_f1_{k}", FLOC)
                nc.sync.dma_start(
                    wt[:], wd["f1"].ap()[128 * k:128 * (k + 1), :])
                for m in range(8):
                    nc.tensor.matmul(ps_f1[m][:],
                                     wt[:, 128 * m:128 * (m + 1)],
                                     x2_t[k][:],
                                     start=(k == 0), stop=(k == ET - 1))
            hT = []
            for m in range(8):
                tg = f"v_{m}" if m < 4 else f"q_{m - 4}"
                h = acts.tile([128, nrem], F32R, name=f"hT_{m}", tag=tg)
                nc.scalar.activation(h[:], ps_f1[m][:], AF.Gelu)
                hT.append(h)
            # f2: quarters of output cols; psum group of 8 m-tiles per quarter
            arin4 = dram.tile([E, nrem], F32, name="arin4", tag="arin4")
            arout4 = dram.tile([E, nrem], F32, name="arout4", tag="arout4", addr_space="Shared")
            HK = FLOC // 128  # 8
            for quarter in range(4):
                ps = []
                for mm in range(8):
                    m = 8 * quarter + mm
                    ps.append(pp(f"ps_f2_{m}", nrem))
                for khalf in range(2):
                    wf_t = []
                    for kk in range(4):
                        k = 4 * khalf + kk
                        wt = wtile(f"w_f2_{quarter}_{k}", 1024)
                        nc.sync.dma_start(
                            wt[:],
                            wd["f2"].ap()[128 * k:128 * (k + 1),
                                          1024 * quarter:1024 * (quarter + 1)])
                        wf_t.append(wt)
                    for kk in range(4):
                        k = 4 * khalf + kk
                        for mm in range(8):
                            nc.tensor.matmul(
                                ps[mm][:],
                                wf_t[kk][:, 128 * mm:128 * (mm + 1)],
                                hT[k][:],
                                start=(k == 0), stop=(k == HK - 1))
                for mm in range(8):
                    m = 8 * quarter + mm
                    st = acts.tile([128, nrem], F32, name=f"st_f2_{m}",
                                   tag="stage", bufs=3)
                    if debug:
                        nc.vector.tensor_copy(st[:], ps[mm][:])
                    else:
                        # fold residual: st = x2/NCORES + partial, so the
                        # cross-core sum of st equals x2 + ffn
                        nc.vector.scalar_tensor_tensor(
                            st[:], x2_t[m][:], 1.0 / NCORES, ps[mm][:],
                            ALU.mult, ALU.add)
                    nc.sync.dma_start(arin4[128 * m:128 * (m + 1), :], st[:])
            if debug:
                do_allreduce("4", arin4, arout4)
                rstd3, nmr3 = residual_ln("x3", arout4, x2_t, nrem,
                                          normalize=True,
                                          dump=dbg.get("dbg_x3"))
                ws_sb = acts.tile([128, ET], F32R, name="ws_sb", tag="ws_sb")
                nc.sync.dma_start(ws_sb[:], wd["s"].ap())
                lp = pstat("ps_logit", nrem)
                for k in range(ET):
                    nc.tensor.matmul(lp[:], ws_sb[:, k:k + 1], x2_t[k][:],
                                     start=(k == 0), stop=(k == ET - 1))
                lsb = acts.tile([1, nrem], F32, name="lsb", tag="lsb")
                nc.vector.tensor_copy(lsb[:], lp[:])
                nc.sync.dma_start(logits_d.ap(), lsb[:])
            else:
                # ReduceScatter xsum over feature blocks; local partial
                # stats; tiny AllReduce of [s1, s2, wsdot]; logits via the
                # affine-LN identity.
                EB = E // NCORES  # 512 features per core
                rs4 = dram.tile([EB, nrem], F32, name="rs4", tag="rs4")
                nc.gpsimd.collective_compute(
                    "ReduceScatter", ALU.add, replica_groups=replica,
                    ins=[arin4.opt()], outs=[rs4.opt()])
                wsb_sb = acts.tile([128, EB // 128], F32R, name="wsb_sb",
                                   tag="ws_sb")
                nc.sync.dma_start(wsb_sb[:], wsb_d.ap())
                s1p = pstat("ps_rs1", nrem)
                s2p = pstat("ps_rs2", nrem)
                wsp = pstat("ps_rsw", nrem)
                bts = []
                for k in range(EB // 128):
                    bt = acts.tile([128, nrem], F32R, name=f"rsb_{k}",
                                   tag="arb", bufs=4)
                    nc.gpsimd.dma_start(bt[:], rs4[128 * k:128 * (k + 1), :])
                    bts.append(bt)
                for k in range(EB // 128):
                    nc.tensor.matmul(s1p[:], ones_col[:], bts[k][:],
                                     start=(k == 0), stop=(k == 3))
                    nc.tensor.matmul(wsp[:], wsb_sb[:, k:k + 1], bts[k][:],
                                     start=(k == 0), stop=(k == 3))
                for k in range(EB // 128):
                    sq = acts.tile([128, nrem], F32R, name=f"rssq_{k}",
                                   tag="stage", bufs=3)
                    nc.scalar.square(sq[:], bts[k][:])
                    nc.tensor.matmul(s2p[:], ones_col[:], sq[:],
                                     start=(k == 0), stop=(k == 3))
                s1s = acts.tile([1, nrem], F32, name="s1s", tag="lmean")
                s2s = acts.tile([1, nrem], F32, name="s2s", tag="lvar")
                wss = acts.tile([1, nrem], F32, name="wss", tag="lr0")
                nc.vector.tensor_copy(s1s[:], s1p[:])
                nc.vector.tensor_copy(s2s[:], s2p[:])
                nc.vector.tensor_copy(wss[:], wsp[:])
                arin5 = dram.tile([4, nrem], F32, name="arin5", tag="arin5")
                arout5 = dram.tile([4, nrem], F32, name="arout5",
                                   tag="arout5", addr_space="Shared")
                nc.sync.dma_start(arin5[0:1, :], s1s[:])
                nc.sync.dma_start(arin5[1:2, :], s2s[:])
                nc.sync.dma_start(arin5[2:3, :], wss[:])
                nc.sync.dma_start(arin5[3:4, :], s1s[:])
                nc.gpsimd.collective_compute(
                    "AllReduce", ALU.add, replica_groups=replica,
                    ins=[arin5.opt()], outs=[arout5.opt()])
                g1 = acts.tile([1, nrem], F32, name="g1", tag="aden")
                g2 = acts.tile([1, nrem], F32, name="g2", tag="arec")
                g3 = acts.tile([1, nrem], F32, name="g3", tag="wsd")
                nc.sync.dma_start(g1[:], arout5[0:1, :])
                nc.sync.dma_start(g2[:], arout5[1:2, :])
                nc.sync.dma_start(g3[:], arout5[2:3, :])
                mean = acts.tile([1, nrem], F32, name="mean_l", tag="lmean")
                var = acts.tile([1, nrem], F32, name="var_l", tag="lvar")
                tmpa = acts.tile([1, nrem], F32, name="tmpa_l", tag="ltmp")
                r0 = acts.tile([1, nrem], F32, name="r0_l", tag="lr0")
                nc.scalar.mul(mean[:], g1[:], 1.0 / E)
                nc.scalar.mul(var[:], g2[:], 1.0 / E)
                nc.scalar.square(tmpa[:], mean[:])
                nc.vector.tensor_sub(var[:], var[:], tmpa[:])
                nc.vector.tensor_scalar_add(var[:], var[:], 1e-5)
                nc.scalar.sqrt(tmpa[:], var[:])
                nc.vector.reciprocal(r0[:], tmpa[:])
                nc.vector.tensor_tensor(tmpa[:], r0[:], r0[:], ALU.mult)
                nc.vector.tensor_tensor(tmpa[:], tmpa[:], var[:], ALU.mult)
                nc.vector.tensor_scalar(tmpa[:], tmpa[:], -0.5, 1.5,
                                        ALU.mult, ALU.add)
                rstd = acts.tile([1, nrem], F32, name="rstd_l", tag="rstd")
                nc.vector.tensor_tensor(rstd[:], r0[:], tmpa[:], ALU.mult)
                nmr = acts.tile([1, nrem], F32, name="nmr_l", tag="nmr")
                nc.vector.scalar_tensor_tensor(nmr[:], mean[:], -1.0,
                                               rstd[:], ALU.mult, ALU.mult)
                wdot = acts.tile([1, nrem], F32, name="wdot", tag="wdot")
                nc.vector.tensor_tensor(wdot[:], rstd[:], g3[:], ALU.mult)
                lsb = acts.tile([1, nrem], F32, name="lsb", tag="lsb")
                nc.vector.scalar_tensor_tensor(lsb[:], nmr[:],
                                               masks[0:1, 2:3], wdot[:],
                                               ALU.mult, ALU.add)
                nc.sync.dma_start(logits_d.ap(), lsb[:])

    nc.compile()
    return nc


# ----------------------------------------------------------------------------
# host orchestration
# ----------------------------------------------------------------------------

def _prep_in_maps(vision_feature, text_embed, sel_idx, rem_idx, ncat, nrem,
                  Wqkv1, Wo1, Wqkv2, Wo2, Wqkvc, Woc, Wf1, Wf2, Ws):
    f32 = np.float32
    sel = vision_feature[sel_idx]
    rem = vision_feature[rem_idx]
    cat = np.concatenate([sel, text_embed], axis=0)
    catT = np.zeros((E, ncat), f32)
    catT[:, :cat.shape[0]] = cat.T
    remT = np.zeros((E, nrem), f32)
    remT[:, :rem.shape[0]] = rem.T

    ncat_real = cat.shape[0]
    nrem_real = rem.shape[0]
    masks = np.zeros((128, 4), f32)
    masks[:ncat_real - 128 * (ncat // 128 - 1), 0] = 1.0
    masks[:nrem_real - 128 * (nrem // 128 - 1), 1] = 1.0
    masks[0, 2] = Ws.astype(np.float64).sum()

    in_maps = []
    for c in range(NCORES):
        hs = slice(DLOC * c, DLOC * (c + 1))
        fs = slice(FLOC * c, FLOC * (c + 1))
        eb = E // NCORES
        m = {"catT": catT, "remT": remT, "masks": masks,
             "ws": np.ascontiguousarray(Ws[0].reshape(ET, 128).T),
             "wsb": np.ascontiguousarray(
                 Ws[0, eb * c:eb * (c + 1)].reshape(eb // 128, 128).T)}
        for l, Wqkv, Wo in (("1", Wqkv1, Wo1), ("2", Wqkv2, Wo2),
                            ("c", Wqkvc, Woc)):
            Wq, Wk, Wv = Wqkv[:E], Wqkv[E:2 * E], Wqkv[2 * E:]
            m["wq" + l] = np.ascontiguousarray(Wq[hs].T)
            m["wk" + l] = np.ascontiguousarray(Wk[hs].T)
            m["wv" + l] = np.ascontiguousarray(Wv[hs].T)
            m["wo" + l] = np.ascontiguousarray(Wo[:, hs].T)
        m["wf1"] = np.ascontiguousarray(Wf1[fs].T)
        m["wf2"] = np.ascontiguousarray(Wf2[:, fs].T)
        in_maps.append(m)
    return in_maps


def run_device(in_maps, ncat_real, nrem_real, debug=False, trace=False):
    from concourse.bass_utils import run_bass_kernel_spmd

    key = (ncat_real, nrem_real, debug)
    if key not in _CACHE:
        _CACHE[key] = _build_device(ncat_real, nrem_real, debug=debug)
    nc = _CACHE[key]
    return run_bass_kernel_spmd(nc, in_maps, list(range(NCORES)), trace=trace)


def _kernel_impl(inputs, debug=False, trace=False):
    vision_feature = np.asarray(inputs["vision_feature"], np.float32)
    text_embed = np.asarray(inputs["text_embed"], np.float32)
    attention_mask = np.asarray(inputs["attention_mask"])

    biases_zero = all(
        not np.any(np.asarray(inputs[b]))
        for b in ("bqkv1", "bo1", "bqkv2", "bo2", "bqkvc", "boc",
                  "bf1", "bf2", "bs"))
    if (not bool(attention_mask.all())) or (not biases_zero):
        return _reference_np(**{k: np.asarray(v) for k, v in inputs.items()}), None

    t, sel_idx, rem_idx = _score_partition(vision_feature, text_embed,
                                           attention_mask)
    ncat_real = t + text_embed.shape[0]
    nrem_real = vision_feature.shape[0] - t
    kk = int(t * EXPAND)

    in_maps = _prep_in_maps(
        vision_feature, text_embed, sel_idx, rem_idx,
        _pad128(ncat_real), _pad128(nrem_real),
        np.asarray(inputs["Wqkv1"], np.float32),
        np.asarray(inputs["Wo1"], np.float32),
        np.asarray(inputs["Wqkv2"], np.float32),
        np.asarray(inputs["Wo2"], np.float32),
        np.asarray(inputs["Wqkvc"], np.float32),
        np.asarray(inputs["Woc"], np.float32),
        np.asarray(inputs["Wf1"], np.float32),
        np.asarray(inputs["Wf2"], np.float32),
        np.asarray(inputs["Ws"], np.float32))
    res = run_device(in_maps, ncat_real, nrem_real, debug=debug, trace=trace)
    logits = res.results[0]["logits"][0, :nrem_real]
    es = (1.0 / (1.0 + np.exp(-logits.astype(np.float32))))
    ei = np.argsort(-es, kind="stable")[:kk]
    final = np.sort(np.concatenate([sel_idx, rem_idx[ei]]))
    return vision_feature[final], res


def kernel(**inputs):
    out, _ = _kernel_impl(inputs)
    return out



# revision 8
# speedup vs baseline: 1.2602x; 1.2602x over previous
"""Trainium2 Bass kernel for nn_CosSimRouter_learn_49778670960796.

Host: cosine-similarity scoring / sort / gather (tiny, shape-determining).
Device (8 NeuronCores, tensor-parallel over heads/hidden):
  3x MHA + FFN + logits. fp16 weights/activations (halves HBM traffic vs
  fp32; ~2e-4 rounding is far below the ~1e-2 top-k selection margin),
  fp32 PSUM accumulation and LN/softmax statistics. Exact token counts
  (no 128-padding of the token free dim). Host-packed weight layouts so
  each weight streams in as a few large DMAs. Collectives: fp16 ARs for
  the three residual streams, fp16 ReduceScatter for the final-LN s2
  stat, and one tiny fp32 AR carrying the linear stats (s1 via
  host-precomputed colsum(Wf2), Ws-dot via host-precomputed Wf2^T Ws).
Host: top-k + final gather (exact rows of the input).
"""

import numpy as np

E = 4096
H = 16
HID = 8192
GAMMA = 0.2
TEMP = 0.05
EXPAND = 0.7
NCORES = 8
ET = E // 128  # 32 feature tiles
DH = E // H  # 256
HL = H // NCORES  # 2 heads per core
DLOC = HL * DH  # 512 local head dims
FLOC = HID // NCORES  # 1024 local ffn hidden
KG = 8  # k-blocks per weight/act chunk

_CACHE = {}


# ----------------------------------------------------------------------------
# host-side reference math (numpy, fp32) for the scoring stage + fallback
# ----------------------------------------------------------------------------

def _score_partition(vision_feature, text_embed, attention_mask):
    vf = vision_feature.astype(np.float32)
    te = text_embed.astype(np.float32)
    vn = vf / np.maximum(np.linalg.norm(vf, axis=-1, keepdims=True), 1e-8)
    tn = te / np.maximum(np.linalg.norm(te, axis=-1, keepdims=True), 1e-8)
    cs = vn @ tn.T
    cs = np.where(attention_mask[None, :], cs, np.float32(0.0))
    m = cs.max(axis=-1) / np.float32(TEMP)
    e = np.exp(m - m.max())
    scores = e / e.sum()
    order = np.argsort(-scores, kind="stable")
    cum = np.cumsum(scores[order])
    t = int((cum <= GAMMA).sum())
    return t, order[:t], order[t:]


def _ln_np(x):
    m = x.mean(-1, keepdims=True)
    v = ((x - m) ** 2).mean(-1, keepdims=True)
    return (x - m) / np.sqrt(v + 1e-5)


def _gelu_np(x):
    import math

    erf = np.frompyfunc(math.erf, 1, 1)
    return (x * 0.5 * (1.0 + erf(x / math.sqrt(2.0)).astype(np.float64))
            ).astype(x.dtype)


def _mha_np(q_in, kv_in, Wqkv, bqkv, Wo, bo):
    dh = E // H
    Wq, Wk, Wv = np.split(Wqkv, 3, axis=0)
    bq, bk, bv = np.split(bqkv, 3)
    q = (q_in @ Wq.T + bq).reshape(-1, H, dh)
    k = (kv_in @ Wk.T + bk).reshape(-1, H, dh)
    v = (kv_in @ Wv.T + bv).reshape(-1, H, dh)
    att = np.einsum("qhd,khd->hqk", q, k) / np.float32(np.sqrt(dh))
    att = att - att.max(-1, keepdims=True)
    att = np.exp(att)
    att /= att.sum(-1, keepdims=True)
    o = np.einsum("hqk,khd->qhd", att.astype(np.float32), v).reshape(-1, E)
    return o @ Wo.T + bo


def _reference_np(vision_feature, text_embed, attention_mask,
                  Wqkv1, bqkv1, Wo1, bo1, Wqkv2, bqkv2, Wo2, bo2,
                  Wqkvc, bqkvc, Woc, boc, Wf1, bf1, Wf2, bf2, Ws, bs):
    t, sel_idx, rem_idx = _score_partition(vision_feature, text_embed,
                                           attention_mask)
    sel = vision_feature[sel_idx]
    rem = vision_feature[rem_idx]
    cat = np.concatenate([sel, text_embed], axis=0)
    x = _ln_np(_mha_np(cat, cat, Wqkv1, bqkv1, Wo1, bo1) + cat)
    r = _ln_np(_mha_np(rem, rem, Wqkv2, bqkv2, Wo2, bo2) + rem)
    x = _ln_np(_mha_np(r, x, Wqkvc, bqkvc, Woc, boc) + r)
    ffn = _gelu_np(x @ Wf1.T + bf1) @ Wf2.T + bf2
    x = _ln_np(x + ffn)
    logits = (x @ Ws.T + bs).squeeze(-1)
    es = 1.0 / (1.0 + np.exp(-logits))
    k = int(t * EXPAND)
    ei = np.argsort(-es, kind="stable")[:k]
    final = np.sort(np.concatenate([sel_idx, rem_idx[ei]]))
    return vision_feature[final]


# ----------------------------------------------------------------------------
# device program
# ----------------------------------------------------------------------------

def _build_device(ncat, nrem, dumps=False):
    import concourse.bacc as bacc
    import concourse.mybir as mybir
    import concourse.tile as tile

    dt = mybir.dt
    F32 = dt.float32
    F16 = dt.float16
    AF = mybir.ActivationFunctionType
    ALU = mybir.AluOpType

    JC = (ncat + 127) // 128  # kv partition tiles for cat (2)
    JR = (nrem + 127) // 128  # kv partition tiles for rem (4)

    nc = bacc.Bacc("TRN2", target_bir_lowering=False, debug=False,
                   num_devices=NCORES)

    # ---------------- DRAM I/O (all host-packed, see _prep_in_maps) --------
    remp_d = nc.dram_tensor("remp", [128, ET * nrem], F16, kind="ExternalInput")
    catp_d = nc.dram_tensor("catp", [128, ET * ncat], F16, kind="ExternalInput")
    wd = {}
    for l in ("1", "2", "c"):
        for p in ("q", "k", "v"):
            wd[p + l] = nc.dram_tensor(f"w{p}{l}", [128, ET * DLOC], F16,
                                       kind="ExternalInput")
        wd["o" + l] = nc.dram_tensor(f"wo{l}", [128, (DLOC // 128) * E], F16,
                                     kind="ExternalInput")
    wd["f1"] = nc.dram_tensor("wf1", [128, ET * FLOC], F16,
                              kind="ExternalInput")
    wd["f2"] = nc.dram_tensor("wf2", [128, (FLOC // 128) * E], F16,
                              kind="ExternalInput")
    ws_d = nc.dram_tensor("wsp", [128, ET], F16, kind="ExternalInput")
    c2w_d = nc.dram_tensor("c2w", [128, 2 * (FLOC // 128)], F16,
                           kind="ExternalInput")
    consts_d = nc.dram_tensor("consts", [1, 2], F32, kind="ExternalInput")
    logits_d = nc.dram_tensor("logits", [1, nrem], F32, kind="ExternalOutput")
    dbg = {}
    if dumps:
        for nm, L in (("dbg_x1", ncat), ("dbg_r", nrem), ("dbg_x2", nrem)):
            dbg[nm] = nc.dram_tensor(nm, [128, ET * L], F16,
                                     kind="ExternalOutput")

    replica = [list(range(NCORES))]
    NG_R = ET // KG  # 4 act groups for rem
    NG_C = ET // KG  # 4 act groups for cat

    with tile.TileContext(nc, num_cores=NCORES) as tc:
        with (
            tc.tile_pool(name="acts", bufs=1) as acts,
            tc.tile_pool(name="psum", bufs=1, space="PSUM") as psum,
            tc.tile_pool(name="dram", bufs=1, space="DRAM") as dram,
        ):
            # ---- constants ----
            ones_col = acts.tile([128, 1], F16, name="ones_col",
                                 tag="ones_col")
            nc.vector.memset(ones_col[:], 1.0)
            ones_row = acts.tile([1, 128], F16, name="ones_row",
                                 tag="ones_row")
            nc.vector.memset(ones_row[:], 1.0)
            ws_sb = acts.tile([128, ET], F16, name="ws_sb", tag="ws_sb")
            nc.sync.dma_start(ws_sb[:], ws_d.ap())
            c2w_sb = acts.tile([128, 2 * (FLOC // 128)], F16, name="c2w_sb",
                               tag="c2w_sb")
            nc.sync.dma_start(c2w_sb[:], c2w_d.ap())
            consts = acts.tile([1, 2], F32, name="consts", tag="consts")
            nc.sync.dma_start(consts[:], consts_d.ap())

            def pp(name, L, parts=128):
                t_ = psum.tile([128, L], F32, name=name, tag="pp", bufs=8)
                return t_[0:parts, :] if parts < 128 else t_[:]

            def pstat(name, L):
                return psum.tile([1, L], F32, name=name, tag="pp", bufs=8)[:]

            def wchunk(name, cols):
                return acts.tile([128, cols], F16, name=name, tag="wt",
                                 bufs=3, padded_shape=[128, KG * FLOC])

            # ---- activations: group tiles + slice helper ----
            def load_x(name, dram_t, L, ngroups):
                ts = []
                for g in range(ngroups):
                    xt = acts.tile([128, KG * L], F16, name=f"{name}_{g}",
                                   tag=f"{name}_{g}")
                    nc.sync.dma_start(
                        xt[:], dram_t.ap()[:, KG * L * g:KG * L * (g + 1)])
                    ts.append(xt)
                return ts

            def xs(ts, L, k):
                g, kk = k // KG, k % KG
                return ts[g][:, kk * L:(kk + 1) * L]

            # ---------------- building blocks ----------------
            def proj_fm(tagbase, w_dram, x_ts, L, outtag):
                """q/k projection -> 4 tiles [128, L] fp16 (DLOC, L) layout."""
                chunks = []
                for g in range(ET // KG):
                    wt = wchunk(f"w_{tagbase}_{g}", KG * DLOC)
                    nc.sync.dma_start(
                        wt[:],
                        w_dram.ap()[:, KG * DLOC * g:KG * DLOC * (g + 1)])
                    chunks.append(wt)
                ps = [pp(f"ps_{tagbase}_{m}", L) for m in range(4)]
                for k in range(ET):
                    g, kk = k // KG, k % KG
                    for m in range(4):
                        nc.tensor.matmul(
                            ps[m],
                            chunks[g][:, kk * DLOC + 128 * m:
                                      kk * DLOC + 128 * (m + 1)],
                            xs(x_ts, L, k),
                            start=(k == 0), stop=(k == ET - 1))
                outs = []
                for m in range(4):
                    o = acts.tile([128, L], F16, name=f"{tagbase}_{m}",
                                  tag=f"{outtag}_{m}")
                    nc.scalar.copy(o[:], ps[m])
                    outs.append(o)
                return outs

            def proj_tm(tagbase, w_dram, x_ts, L, JT):
                """v projection -> JT tiles [128, DLOC] fp16 (kv, DLOC)."""
                chunks = []
                for g in range(ET // KG):
                    wt = wchunk(f"w_{tagbase}_{g}", KG * DLOC)
                    nc.sync.dma_start(
                        wt[:],
                        w_dram.ap()[:, KG * DLOC * g:KG * DLOC * (g + 1)])
                    chunks.append(wt)
                ps = []
                for j in range(JT):
                    pj = min(128, L - 128 * j)
                    ps.append(pp(f"ps_{tagbase}_{j}", DLOC, parts=pj))
                for k in range(ET):
                    g, kk = k // KG, k % KG
                    for j in range(JT):
                        pj = min(128, L - 128 * j)
                        nc.tensor.matmul(
                            ps[j],
                            xs(x_ts, L, k)[:, 128 * j:128 * j + pj],
                            chunks[g][:, kk * DLOC:(kk + 1) * DLOC],
                            start=(k == 0), stop=(k == ET - 1))
                outs = []
                for j in range(JT):
                    pj = min(128, L - 128 * j)
                    o = acts.tile([128, DLOC], F16, name=f"{tagbase}_{j}",
                                  tag=f"v_{j}")
                    nc.scalar.copy(o[0:pj, :], ps[j])
                    outs.append(o)
                return outs

            def attention(tag, qT, kT, vT, Lq, Lkv, JT):
                oT = []
                for h in range(HL):
                    exps = []
                    for j in range(JT):
                        pj = min(128, Lkv - 128 * j)
                        p = pp(f"ps_s_{tag}_{h}_{j}", Lq, parts=pj)
                        for c in range(2):
                            nc.tensor.matmul(
                                p,
                                kT[2 * h + c][:, 128 * j:128 * j + pj],
                                qT[2 * h + c][:],
                                start=(c == 0), stop=(c == 1))
                        e = acts.tile([128, Lq], F16,
                                      name=f"es_{tag}_{h}_{j}",
                                      tag=f"expS_{j}")
                        nc.scalar.activation(e[0:pj, :], p, AF.Exp,
                                             scale=float(1.0 / np.sqrt(DH)))
                        exps.append(e)
                    dsum = pstat(f"ps_d_{tag}_{h}", Lq)
                    for j in range(JT):
                        pj = min(128, Lkv - 128 * j)
                        nc.tensor.matmul(dsum, ones_col[0:pj, :],
                                         exps[j][0:pj, :],
                                         start=(j == 0), stop=(j == JT - 1))
                    den = acts.tile([1, Lq], F32, name=f"den_{tag}_{h}",
                                    tag="aden")
                    rec = acts.tile([1, Lq], F32, name=f"rec_{tag}_{h}",
                                    tag="arec")
                    nc.vector.tensor_copy(den[:], dsum)
                    nc.vector.reciprocal(rec[:], den[:])
                    nc.vector.tensor_tensor(den[:], den[:], rec[:], ALU.mult)
                    nc.vector.tensor_scalar(den[:], den[:], -1.0, 2.0,
                                            ALU.mult, ALU.add)
                    rec2 = acts.tile([1, Lq], F16, name=f"rec2_{tag}_{h}",
                                     tag="rec2")
                    nc.vector.tensor_tensor(rec2[:], rec[:], den[:], ALU.mult)
                    rrep_p = pp(f"ps_rr_{tag}_{h}", Lq)
                    nc.tensor.matmul(rrep_p, ones_row[:], rec2[:],
                                     start=True, stop=True)
                    rrep = acts.tile([128, Lq], F32, name=f"rr_{tag}_{h}",
                                     tag="rrep")
                    nc.scalar.copy(rrep[:], rrep_p)
                    for c in range(2):
                        po = pp(f"ps_o_{tag}_{h}_{c}", Lq)
                        for j in range(JT):
                            pj = min(128, Lkv - 128 * j)
                            nc.tensor.matmul(
                                po,
                                vT[j][0:pj, 256 * h + 128 * c:
                                      256 * h + 128 * (c + 1)],
                                exps[j][0:pj, :],
                                start=(j == 0), stop=(j == JT - 1))
                        o = acts.tile([128, Lq], F16,
                                      name=f"oT_{tag}_{h}_{c}",
                                      tag=f"oT_{2 * h + c}")
                        nc.vector.tensor_tensor(o[:], po, rrep[:], ALU.mult)
                        oT.append(o)
                return oT

            def out_proj_to_dram(tag, oT, w_dram, ar_in, Lq):
                """4 quarter chunks; psum group of 8 m-tiles per quarter;
                one staging tile + one DMA per quarter."""
                for q in range(4):
                    wt = wchunk(f"wo_{tag}_{q}", 4 * 1024)
                    nc.sync.dma_start(
                        wt[:], w_dram.ap()[:, 4096 * q:4096 * (q + 1)])
                    ps = [pp(f"ps_op_{tag}_{q}_{mm}", Lq) for mm in range(8)]
                    for k in range(4):
                        for mm in range(8):
                            nc.tensor.matmul(
                                ps[mm],
                                wt[:, 1024 * k + 128 * mm:
                                   1024 * k + 128 * (mm + 1)],
                                oT[k][:],
                                start=(k == 0), stop=(k == 3))
                    st = acts.tile([128, 8 * Lq], F16, name=f"st_{tag}_{q}",
                                   tag="stage", bufs=2,
                                   padded_shape=[128, 8 * 512])
                    for mm in range(8):
                        nc.vector.tensor_copy(
                            st[:, mm * Lq:(mm + 1) * Lq], ps[mm])
                    nc.sync.dma_start(
                        ar_in[:, 8 * Lq * q:8 * Lq * (q + 1)], st[:])

            def do_allreduce(ar_in, ar_out):
                nc.gpsimd.collective_compute(
                    "AllReduce", ALU.add, replica_groups=replica,
                    ins=[ar_in.opt()], outs=[ar_out.opt()])

            def ln_apply(x_ts, L, Asb, Bsb):
                for k in range(ET):
                    nc.vector.tensor_tensor(xs(x_ts, L, k), xs(x_ts, L, k),
                                            Asb[:], ALU.mult)
                    nc.vector.tensor_tensor(xs(x_ts, L, k), xs(x_ts, L, k),
                                            Bsb[:], ALU.add)

            def residual_ln2(tag, ar_out, x_ts, L, dump=None):
                s1p = pstat(f"ps_s1_{tag}", L)
                s2p = pstat(f"ps_s2_{tag}", L)
                for g in range(ET // KG):
                    b = acts.tile([128, KG * L], F16, name=f"arb_{tag}_{g}",
                                  tag="arb", bufs=2,
                                  padded_shape=[128, KG * 512])
                    nc.sync.dma_start(
                        b[:], ar_out[:, KG * L * g:KG * L * (g + 1)])
                    nc.vector.tensor_tensor(x_ts[g][:], b[:], x_ts[g][:],
                                            ALU.add)
                for k in range(ET):
                    nc.tensor.matmul(s1p, ones_col[:], xs(x_ts, L, k),
                                     start=(k == 0), stop=(k == ET - 1))
                for k in range(ET):
                    sq = acts.tile([128, L], F16, name=f"sq_{tag}_{k}",
                                   tag="sqt", bufs=3, padded_shape=[128, 512])
                    nc.scalar.square(sq[:], xs(x_ts, L, k))
                    nc.tensor.matmul(s2p, ones_col[:], sq[:],
                                     start=(k == 0), stop=(k == ET - 1))
                mean = acts.tile([1, L], F32, name=f"mean_{tag}", tag="lmean")
                var = acts.tile([1, L], F32, name=f"var_{tag}", tag="lvar")
                tmpa = acts.tile([1, L], F32, name=f"tmpa_{tag}", tag="ltmp")
                r0 = acts.tile([1, L], F32, name=f"r0_{tag}", tag="lr0")
                nc.scalar.mul(mean[:], s1p, 1.0 / E)
                nc.scalar.mul(var[:], s2p, 1.0 / E)
                nc.scalar.square(tmpa[:], mean[:])
                nc.vector.tensor_sub(var[:], var[:], tmpa[:])
                nc.vector.tensor_scalar_add(var[:], var[:], 1e-5)
                nc.scalar.sqrt(tmpa[:], var[:])
                nc.vector.reciprocal(r0[:], tmpa[:])
                nc.vector.tensor_tensor(tmpa[:], r0[:], r0[:], ALU.mult)
                nc.vector.tensor_tensor(tmpa[:], tmpa[:], var[:], ALU.mult)
                nc.vector.tensor_scalar(tmpa[:], tmpa[:], -0.5, 1.5, ALU.mult,
                                        ALU.add)
                rstd = acts.tile([1, L], F16, name=f"rstd_{tag}", tag="rstd")
                nmr = acts.tile([1, L], F16, name=f"nmr_{tag}", tag="nmr")
                nc.vector.tensor_tensor(rstd[:], r0[:], tmpa[:], ALU.mult)
                nc.vector.scalar_tensor_tensor(nmr[:], mean[:], -1.0, rstd[:],
                                               ALU.mult, ALU.mult)
                Apsum = pp(f"ps_A_{tag}", L)
                nc.tensor.matmul(Apsum, ones_row[:], rstd[:], start=True,
                                 stop=True)
                Bpsum = pp(f"ps_B_{tag}", L)
                nc.tensor.matmul(Bpsum, ones_row[:], nmr[:], start=True,
                                 stop=True)
                Asb = acts.tile([128, L], F16, name=f"A_{tag}", tag="Asb")
                nc.scalar.copy(Asb[:], Apsum)
                Bsb = acts.tile([128, L], F16, name=f"B_{tag}", tag="Bsb")
                nc.scalar.copy(Bsb[:], Bpsum)
                ln_apply(x_ts, L, Asb, Bsb)
                if dump is not None:
                    for g in range(ET // KG):
                        nc.sync.dma_start(
                            dump.ap()[:, KG * L * g:KG * L * (g + 1)],
                            x_ts[g][:])

            # ================= program =================
            rem_ts = load_x("remx", remp_d, nrem, NG_R)
            cat_ts = load_x("catx", catp_d, ncat, NG_C)

            # ---- MHA2 (rem self-attention) ----
            q2 = proj_fm("q2", wd["q2"], rem_ts, nrem, "q")
            k2 = proj_fm("k2", wd["k2"], rem_ts, nrem, "k")
            v2 = proj_tm("v2", wd["v2"], rem_ts, nrem, JR)
            o2 = attention("a2", q2, k2, v2, nrem, nrem, JR)
            arin2 = dram.tile([128, ET * nrem], F16, name="arin2", tag="arin2")
            arout2 = dram.tile([128, ET * nrem], F16, name="arout2",
                               tag="arout2", addr_space="Shared")
            out_proj_to_dram("op2", o2, wd["o2"], arin2, nrem)
            do_allreduce(arin2, arout2)

            # ---- MHA1 (cat self-attention), overlaps AR2 ----
            q1 = proj_fm("q1", wd["q1"], cat_ts, ncat, "q")
            k1 = proj_fm("k1", wd["k1"], cat_ts, ncat, "k")
            v1 = proj_tm("v1", wd["v1"], cat_ts, ncat, JC)
            o1 = attention("a1", q1, k1, v1, ncat, ncat, JC)
            arin1 = dram.tile([128, ET * ncat], F16, name="arin1", tag="arin1")
            arout1 = dram.tile([128, ET * ncat], F16, name="arout1",
                               tag="arout1", addr_space="Shared")
            out_proj_to_dram("op1", o1, wd["o1"], arin1, ncat)
            do_allreduce(arin1, arout1)

            # ---- LN stages: r = LN(AR2 + rem); x1 = LN(AR1 + cat) ----
            residual_ln2("r", arout2, rem_ts, nrem, dump=dbg.get("dbg_r"))
            residual_ln2("x1", arout1, cat_ts, ncat, dump=dbg.get("dbg_x1"))

            # ---- MHAc (q from r, kv from x1) ----
            qc = proj_fm("qc", wd["qc"], rem_ts, nrem, "q")
            kc = proj_fm("kc", wd["kc"], cat_ts, ncat, "k")
            vc = proj_tm("vc", wd["vc"], cat_ts, ncat, JC)
            oc = attention("ac", qc, kc, vc, nrem, ncat, JC)
            arinc = dram.tile([128, ET * nrem], F16, name="arinc", tag="arinc")
            aroutc = dram.tile([128, ET * nrem], F16, name="aroutc",
                               tag="aroutc", addr_space="Shared")
            out_proj_to_dram("opc", oc, wd["oc"], arinc, nrem)
            do_allreduce(arinc, aroutc)
            residual_ln2("x2", aroutc, rem_ts, nrem, dump=dbg.get("dbg_x2"))

            # ---- linear logit stat: Ws . x2 ----
            wxp = pstat("ps_wx2", nrem)
            for k in range(ET):
                nc.tensor.matmul(wxp, ws_sb[:, k:k + 1],
                                 xs(rem_ts, nrem, k),
                                 start=(k == 0), stop=(k == ET - 1))
            wx2 = acts.tile([1, nrem], F32, name="wx2", tag="wx2")
            nc.vector.tensor_copy(wx2[:], wxp)

            # ---- FFN f1: hT = gelu(Wf1_shard @ x2) ----
            f1chunks = []
            for g in range(ET // KG):
                wt = wchunk(f"w_f1_{g}", KG * FLOC)
                nc.sync.dma_start(
                    wt[:], wd["f1"].ap()[:, KG * FLOC * g:KG * FLOC * (g + 1)])
                f1chunks.append(wt)
            ps_f1 = [pp(f"ps_f1_{m}", nrem) for m in range(8)]
            for k in range(ET):
                g, kk = k // KG, k % KG
                for m in range(8):
                    nc.tensor.matmul(
                        ps_f1[m],
                        f1chunks[g][:, kk * FLOC + 128 * m:
                                    kk * FLOC + 128 * (m + 1)],
                        xs(rem_ts, nrem, k),
                        start=(k == 0), stop=(k == ET - 1))
            hT = []
            for m in range(8):
                h = acts.tile([128, nrem], F16, name=f"hT_{m}", tag=f"hT_{m}")
                nc.scalar.activation(h[:], ps_f1[m], AF.Gelu)
                hT.append(h)

            # ---- linear logit stats from hT: s1 = c2.g ; ws += w2s.g/256 ----
            c2p = pstat("ps_c2", nrem)
            w2p = pstat("ps_w2s", nrem)
            for m in range(8):
                nc.tensor.matmul(c2p, c2w_sb[:, m:m + 1], hT[m][:],
                                 start=(m == 0), stop=(m == 7))
            for m in range(8):
                nc.tensor.matmul(w2p, c2w_sb[:, 8 + m:9 + m], hT[m][:],
                                 start=(m == 0), stop=(m == 7))
            s1part = acts.tile([1, nrem], F32, name="s1part", tag="s1part")
            wspart = acts.tile([1, nrem], F32, name="wspart", tag="wspart")
            nc.vector.tensor_copy(s1part[:], c2p)
            nc.vector.tensor_scalar(wx2[:], wx2[:], 1.0 / NCORES, 0.0,
                                    ALU.mult, ALU.add)
            nc.vector.scalar_tensor_tensor(wspart[:], w2p, 1.0 / 256.0,
                                           wx2[:], ALU.mult, ALU.add)
            arin6 = dram.tile([4, nrem], F32, name="arin6", tag="arin6")
            arout6 = dram.tile([4, nrem], F32, name="arout6", tag="arout6",
                               addr_space="Shared")
            nc.sync.dma_start(arin6[0:1, :], s1part[:])
            nc.sync.dma_start(arin6[1:2, :], wspart[:])
            nc.sync.dma_start(arin6[3:4, :], s1part[:])

            # ---- FFN f2: partial = x2/8 + Wf2_shard^T hT, to RS ----
            arin4 = dram.tile([128, ET * nrem], F16, name="arin4", tag="arin4")
            rs4 = dram.tile([16, ET * nrem], F16, name="rs4", tag="rs4")
            for q in range(4):
                wt = wchunk(f"w_f2_{q}", 8 * 1024)
                nc.sync.dma_start(
                    wt[:], wd["f2"].ap()[:, 8192 * q:8192 * (q + 1)])
                ps = [pp(f"ps_f2_{q}_{mm}", nrem) for mm in range(8)]
                for k in range(8):
                    for mm in range(8):
                        nc.tensor.matmul(
                            ps[mm],
                            wt[:, 1024 * k + 128 * mm:
                               1024 * k + 128 * (mm + 1)],
                            hT[k][:],
                            start=(k == 0), stop=(k == 7))
                st = acts.tile([128, 8 * nrem], F16, name=f"st_f2_{q}",
                               tag="stage", bufs=2,
                               padded_shape=[128, 8 * 512])
                for mm in range(8):
                    m = 8 * q + mm
                    nc.vector.scalar_tensor_tensor(
                        st[:, mm * nrem:(mm + 1) * nrem],
                        xs(rem_ts, nrem, m), 1.0 / NCORES, ps[mm],
                        ALU.mult, ALU.add)
                nc.sync.dma_start(
                    arin4[:, 8 * nrem * q:8 * nrem * (q + 1)], st[:])
            nc.gpsimd.collective_compute(
                "ReduceScatter", ALU.add, replica_groups=replica,
                ins=[arin4.opt()], outs=[rs4.opt()])

            # ---- s2 from the scattered summed features ----
            NCH = 8
            CW = ET * nrem // NCH  # columns per rs4 read chunk
            NSUB = CW // nrem  # k-subblocks per chunk
            s2p = pstat("ps_rs2", nrem)
            for cch in range(NCH):
                bt = acts.tile([16, CW], F16, name=f"rsb_{cch}", tag="rsb",
                               bufs=2, padded_shape=[16, 4 * 512])
                nc.gpsimd.dma_start(bt[:], rs4[:, CW * cch:CW * (cch + 1)])
                sq = acts.tile([16, CW], F16, name=f"rssq_{cch}", tag="rssq",
                               bufs=2, padded_shape=[16, 4 * 512])
                nc.scalar.square(sq[:], bt[:])
                for s in range(NSUB):
                    k = cch * NSUB + s
                    nc.tensor.matmul(s2p, ones_col[0:16, :],
                                     sq[:, s * nrem:(s + 1) * nrem],
                                     start=(k == 0), stop=(k == ET - 1))
            s2part = acts.tile([1, nrem], F32, name="s2part", tag="s2part")
            nc.vector.tensor_copy(s2part[:], s2p)
            nc.sync.dma_start(arin6[2:3, :], s2part[:])
            do_allreduce(arin6, arout6)

            # ---- final logit: affine-LN identity ----
            g1 = acts.tile([1, nrem], F32, name="g1", tag="aden")
            g2 = acts.tile([1, nrem], F32, name="g2", tag="arec")
            g3 = acts.tile([1, nrem], F32, name="g3", tag="wsd")
            nc.sync.dma_start(g1[:], arout6[0:1, :])
            nc.sync.dma_start(g2[:], arout6[1:2, :])
            nc.sync.dma_start(g3[:], arout6[2:3, :])
            mean = acts.tile([1, nrem], F32, name="mean_l", tag="lmean")
            var = acts.tile([1, nrem], F32, name="var_l", tag="lvar")
            tmpa = acts.tile([1, nrem], F32, name="tmpa_l", tag="ltmp")
            r0 = acts.tile([1, nrem], F32, name="r0_l", tag="lr0")
            nc.scalar.mul(mean[:], g1[:], 1.0 / E)
            nc.scalar.mul(var[:], g3[:], 1.0 / E)
            nc.scalar.square(tmpa[:], mean[:])
            nc.vector.tensor_sub(var[:], var[:], tmpa[:])
            nc.vector.tensor_scalar_add(var[:], var[:], 1e-5)
            nc.scalar.sqrt(tmpa[:], var[:])
            nc.vector.reciprocal(r0[:], tmpa[:])
            nc.vector.tensor_tensor(tmpa[:], r0[:], r0[:], ALU.mult)
            nc.vector.tensor_tensor(tmpa[:], tmpa[:], var[:], ALU.mult)
            nc.vector.tensor_scalar(tmpa[:], tmpa[:], -0.5, 1.5,
                                    ALU.mult, ALU.add)
            rstd = acts.tile([1, nrem], F32, name="rstd_l", tag="rstd")
            nc.vector.tensor_tensor(rstd[:], r0[:], tmpa[:], ALU.mult)
            nmr = acts.tile([1, nrem], F32, name="nmr_l", tag="nmr")
            nc.vector.scalar_tensor_tensor(nmr[:], mean[:], -1.0,
                                           rstd[:], ALU.mult, ALU.mult)
            wdot = acts.tile([1, nrem], F32, name="wdot", tag="wdot")
            nc.vector.tensor_tensor(wdot[:], rstd[:], g2[:], ALU.mult)
            lsb = acts.tile([1, nrem], F32, name="lsb", tag="lsb")
            nc.vector.scalar_tensor_tensor(lsb[:], nmr[:],
                                           consts[0:1, 0:1], wdot[:],
                                           ALU.mult, ALU.add)
            nc.sync.dma_start(logits_d.ap(), lsb[:])

    nc.compile()
    return nc


# ----------------------------------------------------------------------------
# host orchestration
# ----------------------------------------------------------------------------

def _packx(XT):
    """[E, L] fp32 -> [128, ET*L] fp16 feature-block pack."""
    L = XT.shape[1]
    return np.ascontiguousarray(
        XT.reshape(ET, 128, L).transpose(1, 0, 2).reshape(128, ET * L)
        .astype(np.float16))


def _prep_in_maps(vision_feature, text_embed, sel_idx, rem_idx,
                  Wqkv1, Wo1, Wqkv2, Wo2, Wqkvc, Woc, Wf1, Wf2, Ws):
    f16 = np.float16
    sel = vision_feature[sel_idx]
    rem = vision_feature[rem_idx]
    cat = np.concatenate([sel, text_embed], axis=0)

    remp = _packx(np.ascontiguousarray(rem.T))
    catp = _packx(np.ascontiguousarray(cat.T))
    ws_pack = np.ascontiguousarray(Ws[0].reshape(ET, 128).T.astype(f16))
    consts = np.array([[np.float64(Ws.astype(np.float64).sum()), 0.0]],
                      np.float32)

    in_maps = []
    for c in range(NCORES):
        hs = slice(DLOC * c, DLOC * (c + 1))
        fs = slice(FLOC * c, FLOC * (c + 1))
        m = {"remp": remp, "catp": catp, "wsp": ws_pack, "consts": consts}
        for l, Wqkv, Wo in (("1", Wqkv1, Wo1), ("2", Wqkv2, Wo2),
                            ("c", Wqkvc, Woc)):
            Wq, Wk, Wv = Wqkv[:E], Wqkv[E:2 * E], Wqkv[2 * E:]
            for nm, W in (("q", Wq), ("k", Wk), ("v", Wv)):
                A = W[hs].T  # [E, DLOC]
                m[f"w{nm}{l}"] = np.ascontiguousarray(
                    A.reshape(ET, 128, DLOC).transpose(1, 0, 2)
                    .reshape(128, ET * DLOC).astype(f16))
            WoT = Wo[:, hs].T  # [DLOC, E]
            m[f"wo{l}"] = np.ascontiguousarray(
                WoT.reshape(4, 128, 4, 1024).transpose(1, 2, 0, 3)
                .reshape(128, 4 * E).astype(f16))
        A = Wf1[fs].T  # [E, FLOC]
        m["wf1"] = np.ascontiguousarray(
            A.reshape(ET, 128, FLOC).transpose(1, 0, 2)
            .reshape(128, ET * FLOC).astype(f16))
        W2T = Wf2[:, fs].T  # [FLOC, E]
        m["wf2"] = np.ascontiguousarray(
            W2T.reshape(8, 128, 4, 1024).transpose(1, 2, 0, 3)
            .reshape(128, 8 * E).astype(f16))
        c2 = Wf2[:, fs].astype(np.float64).sum(axis=0)  # [FLOC]
        w2s = 256.0 * (Ws[0].astype(np.float64) @ Wf2[:, fs].astype(np.float64))
        c2w = np.concatenate([c2.reshape(8, 128).T, w2s.reshape(8, 128).T],
                             axis=1)  # [128, 16]
        m["c2w"] = np.ascontiguousarray(c2w.astype(f16))
        in_maps.append(m)
    return in_maps


def run_device(in_maps, ncat_real, nrem_real, dumps=False, trace=False):
    from concourse.bass_utils import run_bass_kernel_spmd

    key = (ncat_real, nrem_real, dumps)
    if key not in _CACHE:
        _CACHE[key] = _build_device(ncat_real, nrem_real, dumps=dumps)
    nc = _CACHE[key]
    return run_bass_kernel_spmd(nc, in_maps, list(range(NCORES)), trace=trace)


def _kernel_impl(inputs, debug=False, trace=False):
    vision_feature = np.asarray(inputs["vision_feature"], np.float32)
    text_embed = np.asarray(inputs["text_embed"], np.float32)
    attention_mask = np.asarray(inputs["attention_mask"])

    biases_zero = all(
        not np.any(np.asarray(inputs[b]))
        for b in ("bqkv1", "bo1", "bqkv2", "bo2", "bqkvc", "boc",
                  "bf1", "bf2", "bs"))
    if (not bool(attention_mask.all())) or (not biases_zero):
        return (_reference_np(**{k: np.asarray(v) for k, v in inputs.items()}),
                None)

    t, sel_idx, rem_idx = _score_partition(vision_feature, text_embed,
                                           attention_mask)
    ncat_real = t + text_embed.shape[0]
    nrem_real = vision_feature.shape[0] - t
    kk = int(t * EXPAND)

    in_maps = _prep_in_maps(
        vision_feature, text_embed, sel_idx, rem_idx,
        np.asarray(inputs["Wqkv1"], np.float32),
        np.asarray(inputs["Wo1"], np.float32),
        np.asarray(inputs["Wqkv2"], np.float32),
        np.asarray(inputs["Wo2"], np.float32),
        np.asarray(inputs["Wqkvc"], np.float32),
        np.asarray(inputs["Woc"], np.float32),
        np.asarray(inputs["Wf1"], np.float32),
        np.asarray(inputs["Wf2"], np.float32),
        np.asarray(inputs["Ws"], np.float32))
    res = run_device(in_maps, ncat_real, nrem_real, dumps=debug, trace=trace)
    logits = res.results[0]["logits"][0, :nrem_real]
    es = (1.0 / (1.0 + np.exp(-logits.astype(np.float32))))
    ei = np.argsort(-es, kind="stable")[:kk]
    final = np.sort(np.concatenate([sel_idx, rem_idx[ei]]))
    return vision_feature[final], res


def kernel(**inputs):
    out, _ = _kernel_impl(inputs)
    return out


# revision 28
# speedup vs baseline: 1.3635x; 1.0820x over previous
"""Trainium2 Bass kernel for nn_CosSimRouter_learn_49778670960796.

Host: cosine-similarity scoring / sort / gather (tiny, shape-determining).
Device (8 NeuronCores, tensor-parallel over heads/hidden):
  3x MHA + FFN + logits. fp16 weights/activations (halves HBM traffic vs
  fp32; ~2e-4 rounding is far below the ~1e-2 top-k selection margin),
  fp32 PSUM accumulation and LN/softmax statistics. Exact token counts
  (no 128-padding of the token free dim). Host-packed weight layouts so
  each weight streams in as a few large DMAs. Collectives: fp16 ARs for
  the three residual streams, fp16 ReduceScatter for the final-LN s2
  stat, and one tiny fp32 AR carrying the linear stats (s1 via
  host-precomputed colsum(Wf2), Ws-dot via host-precomputed Wf2^T Ws).
Host: top-k + final gather (exact rows of the input).
"""

import numpy as np

E = 4096
H = 16
HID = 8192
GAMMA = 0.2
TEMP = 0.05
EXPAND = 0.7
NCORES = 8
ET = E // 128  # 32 feature tiles
DH = E // H  # 256
HL = H // NCORES  # 2 heads per core
DLOC = HL * DH  # 512 local head dims
FLOC = HID // NCORES  # 1024 local ffn hidden
KG = 8  # k-blocks per weight/act chunk

_CACHE = {}


# ----------------------------------------------------------------------------
# host-side reference math (numpy, fp32) for the scoring stage + fallback
# ----------------------------------------------------------------------------

def _score_partition(vision_feature, text_embed, attention_mask):
    vf = vision_feature.astype(np.float32)
    te = text_embed.astype(np.float32)
    vn = vf / np.maximum(np.linalg.norm(vf, axis=-1, keepdims=True), 1e-8)
    tn = te / np.maximum(np.linalg.norm(te, axis=-1, keepdims=True), 1e-8)
    cs = vn @ tn.T
    cs = np.where(attention_mask[None, :], cs, np.float32(0.0))
    m = cs.max(axis=-1) / np.float32(TEMP)
    e = np.exp(m - m.max())
    scores = e / e.sum()
    order = np.argsort(-scores, kind="stable")
    cum = np.cumsum(scores[order])
    t = int((cum <= GAMMA).sum())
    return t, order[:t], order[t:]


def _ln_np(x):
    m = x.mean(-1, keepdims=True)
    v = ((x - m) ** 2).mean(-1, keepdims=True)
    return (x - m) / np.sqrt(v + 1e-5)


def _gelu_np(x):
    import math

    erf = np.frompyfunc(math.erf, 1, 1)
    return (x * 0.5 * (1.0 + erf(x / math.sqrt(2.0)).astype(np.float64))
            ).astype(x.dtype)


def _mha_np(q_in, kv_in, Wqkv, bqkv, Wo, bo):
    dh = E // H
    Wq, Wk, Wv = np.split(Wqkv, 3, axis=0)
    bq, bk, bv = np.split(bqkv, 3)
    q = (q_in @ Wq.T + bq).reshape(-1, H, dh)
    k = (kv_in @ Wk.T + bk).reshape(-1, H, dh)
    v = (kv_in @ Wv.T + bv).reshape(-1, H, dh)
    att = np.einsum("qhd,khd->hqk", q, k) / np.float32(np.sqrt(dh))
    att = att - att.max(-1, keepdims=True)
    att = np.exp(att)
    att /= att.sum(-1, keepdims=True)
    o = np.einsum("hqk,khd->qhd", att.astype(np.float32), v).reshape(-1, E)
    return o @ Wo.T + bo


def _reference_np(vision_feature, text_embed, attention_mask,
                  Wqkv1, bqkv1, Wo1, bo1, Wqkv2, bqkv2, Wo2, bo2,
                  Wqkvc, bqkvc, Woc, boc, Wf1, bf1, Wf2, bf2, Ws, bs):
    t, sel_idx, rem_idx = _score_partition(vision_feature, text_embed,
                                           attention_mask)
    sel = vision_feature[sel_idx]
    rem = vision_feature[rem_idx]
    cat = np.concatenate([sel, text_embed], axis=0)
    x = _ln_np(_mha_np(cat, cat, Wqkv1, bqkv1, Wo1, bo1) + cat)
    r = _ln_np(_mha_np(rem, rem, Wqkv2, bqkv2, Wo2, bo2) + rem)
    x = _ln_np(_mha_np(r, x, Wqkvc, bqkvc, Woc, boc) + r)
    ffn = _gelu_np(x @ Wf1.T + bf1) @ Wf2.T + bf2
    x = _ln_np(x + ffn)
    logits = (x @ Ws.T + bs).squeeze(-1)
    es = 1.0 / (1.0 + np.exp(-logits))
    k = int(t * EXPAND)
    ei = np.argsort(-es, kind="stable")[:k]
    final = np.sort(np.concatenate([sel_idx, rem_idx[ei]]))
    return vision_feature[final]


# ----------------------------------------------------------------------------
# device program
# ----------------------------------------------------------------------------

def _build_device(ncat, nrem, dumps=False):
    import concourse.bacc as bacc
    import concourse.mybir as mybir
    import concourse.tile as tile

    dt = mybir.dt
    F32 = dt.float32
    F16 = dt.float16
    AF = mybir.ActivationFunctionType
    ALU = mybir.AluOpType

    JC = (ncat + 127) // 128  # kv partition tiles for cat (2)
    JR = (nrem + 127) // 128  # kv partition tiles for rem (4)

    nc = bacc.Bacc("TRN2", target_bir_lowering=False, debug=False,
                   num_devices=NCORES)

    # ---------------- DRAM I/O (all host-packed, see _prep_in_maps) --------
    remp_d = nc.dram_tensor("remp", [128, ET * nrem], F16, kind="ExternalInput")
    catp_d = nc.dram_tensor("catp", [128, ET * ncat], F16, kind="ExternalInput")
    wd = {}
    for l in ("1", "2", "c"):
        for p in ("q", "k", "v"):
            wd[p + l] = nc.dram_tensor(f"w{p}{l}", [128, ET * DLOC], F16,
                                       kind="ExternalInput")
        wd["o" + l] = nc.dram_tensor(f"wo{l}", [128, (DLOC // 128) * E], F16,
                                     kind="ExternalInput")
    wd["f1"] = nc.dram_tensor("wf1", [128, ET * FLOC], F16,
                              kind="ExternalInput")
    wd["f2"] = nc.dram_tensor("wf2", [128, (FLOC // 128) * E], F16,
                              kind="ExternalInput")
    ws_d = nc.dram_tensor("wsp", [128, ET], F16, kind="ExternalInput")
    c2w_d = nc.dram_tensor("c2w", [128, 2 * (FLOC // 128)], F16,
                           kind="ExternalInput")
    consts_d = nc.dram_tensor("consts", [128, 2], F32, kind="ExternalInput")
    logits_d = nc.dram_tensor("logits", [1, 512], F32, kind="ExternalOutput")
    dbg = {}
    if dumps:
        for nm, L in (("dbg_x1", ncat), ("dbg_r", nrem), ("dbg_x2", nrem)):
            dbg[nm] = nc.dram_tensor(nm, [128, ET * L], F16,
                                     kind="ExternalOutput")

    replica = [list(range(NCORES))]
    NG_R = ET // KG  # 4 act groups for rem
    NG_C = ET // KG  # 4 act groups for cat

    with tile.TileContext(nc, num_cores=NCORES) as tc:
        with (
            tc.tile_pool(name="acts", bufs=1) as acts,
            tc.tile_pool(name="psum", bufs=1, space="PSUM") as psum,
            tc.tile_pool(name="dram", bufs=1, space="DRAM") as dram,
        ):
            # ---- constants ----
            ones_col = acts.tile([128, 1], F16, name="ones_col",
                                 tag="ones_col")
            nc.vector.memset(ones_col[:], 1.0)
            ones_row = acts.tile([1, 128], F16, name="ones_row",
                                 tag="ones_row")
            nc.vector.memset(ones_row[:], 1.0)
            ws_sb = acts.tile([128, ET], F16, name="ws_sb", tag="ws_sb")
            nc.sync.dma_start(ws_sb[:], ws_d.ap())
            c2w_sb = acts.tile([128, 2 * (FLOC // 128)], F16, name="c2w_sb",
                               tag="c2w_sb")
            nc.sync.dma_start(c2w_sb[:], c2w_d.ap())
            consts = acts.tile([128, 2], F32, name="consts", tag="consts")
            nc.sync.dma_start(consts[:], consts_d.ap())

            def pp(name, L, parts=128):
                t_ = psum.tile([128, L], F32, name=name, tag="pp", bufs=8)
                return t_[0:parts, :] if parts < 128 else t_[:]

            def pstat(name, L):
                return psum.tile([1, L], F32, name=name, tag="pp", bufs=8)[:]

            def wchunk(name, cols):
                return acts.tile([128, cols], F16, name=name, tag="wt",
                                 bufs=3, padded_shape=[128, KG * FLOC])

            # ---- activations: group tiles + slice helper ----
            def load_x(name, dram_t, L, ngroups):
                ts = []
                for g in range(ngroups):
                    xt = acts.tile([128, KG * L], F16, name=f"{name}_{g}",
                                   tag=f"{name}_{g}")
                    nc.sync.dma_start(
                        xt[:], dram_t.ap()[:, KG * L * g:KG * L * (g + 1)])
                    ts.append(xt)
                return ts

            def xs(ts, L, k):
                g, kk = k // KG, k % KG
                return ts[g][:, kk * L:(kk + 1) * L]

            # ---------------- building blocks ----------------
            def proj_fm(tagbase, w_dram, x_ts, L, outtag):
                """q/k projection -> 4 tiles [128, L] fp16 (DLOC, L) layout."""
                chunks = []
                for g in range(ET // KG):
                    wt = wchunk(f"w_{tagbase}_{g}", KG * DLOC)
                    nc.sync.dma_start(
                        wt[:],
                        w_dram.ap()[:, KG * DLOC * g:KG * DLOC * (g + 1)])
                    chunks.append(wt)
                ps = [pp(f"ps_{tagbase}_{m}", L) for m in range(4)]
                for k in range(ET):
                    g, kk = k // KG, k % KG
                    for m in range(4):
                        nc.tensor.matmul(
                            ps[m],
                            chunks[g][:, kk * DLOC + 128 * m:
                                      kk * DLOC + 128 * (m + 1)],
                            xs(x_ts, L, k),
                            start=(k == 0), stop=(k == ET - 1))
                outs = []
                for m in range(4):
                    o = acts.tile([128, L], F16, name=f"{tagbase}_{m}",
                                  tag=f"{outtag}_{m}")
                    nc.scalar.copy(o[:], ps[m])
                    outs.append(o)
                return outs

            def proj_tm(tagbase, w_dram, x_ts, L, JT):
                """v projection -> JT tiles [128, DLOC] fp16 (kv, DLOC)."""
                chunks = []
                for g in range(ET // KG):
                    wt = wchunk(f"w_{tagbase}_{g}", KG * DLOC)
                    nc.sync.dma_start(
                        wt[:],
                        w_dram.ap()[:, KG * DLOC * g:KG * DLOC * (g + 1)])
                    chunks.append(wt)
                ps = []
                for j in range(JT):
                    pj = min(128, L - 128 * j)
                    ps.append(pp(f"ps_{tagbase}_{j}", DLOC, parts=pj))
                for k in range(ET):
                    g, kk = k // KG, k % KG
                    for j in range(JT):
                        pj = min(128, L - 128 * j)
                        nc.tensor.matmul(
                            ps[j],
                            xs(x_ts, L, k)[:, 128 * j:128 * j + pj],
                            chunks[g][:, kk * DLOC:(kk + 1) * DLOC],
                            start=(k == 0), stop=(k == ET - 1))
                outs = []
                for j in range(JT):
                    pj = min(128, L - 128 * j)
                    o = acts.tile([128, DLOC], F16, name=f"{tagbase}_{j}",
                                  tag=f"v_{j}")
                    nc.scalar.copy(o[0:pj, :], ps[j])
                    outs.append(o)
                return outs

            def attention(tag, qT, kT, vT, Lq, Lkv, JT):
                # Emission order keeps the PE queue free of stalls: all
                # score/dsum/po matmuls are independent of the softmax
                # denominator chain (DVE), which runs concurrently; the
                # rrep broadcast matmuls come last.
                exps_h = []
                for h in range(HL):
                    exps = []
                    for j in range(JT):
                        pj = min(128, Lkv - 128 * j)
                        p = pp(f"ps_s_{tag}_{h}_{j}", Lq, parts=pj)
                        for c in range(2):
                            nc.tensor.matmul(
                                p,
                                kT[2 * h + c][:, 128 * j:128 * j + pj],
                                qT[2 * h + c][:],
                                start=(c == 0), stop=(c == 1))
                        e = acts.tile([128, Lq], F16,
                                      name=f"es_{tag}_{h}_{j}",
                                      tag=f"expS_{h}_{j}")
                        nc.scalar.activation(e[0:pj, :], p, AF.Exp,
                                             scale=float(1.0 / np.sqrt(DH)))
                        exps.append(e)
                    exps_h.append(exps)
                rec2s = []
                for h in range(HL):
                    dsum = pstat(f"ps_d_{tag}_{h}", Lq)
                    for j in range(JT):
                        pj = min(128, Lkv - 128 * j)
                        nc.tensor.matmul(dsum, ones_col[0:pj, :],
                                         exps_h[h][j][0:pj, :],
                                         start=(j == 0), stop=(j == JT - 1))
                    den = acts.tile([1, Lq], F32, name=f"den_{tag}_{h}",
                                    tag="aden")
                    rec = acts.tile([1, Lq], F32, name=f"rec_{tag}_{h}",
                                    tag="arec")
                    nc.vector.tensor_copy(den[:], dsum)
                    nc.vector.reciprocal(rec[:], den[:])
                    nc.vector.tensor_tensor(den[:], den[:], rec[:], ALU.mult)
                    nc.vector.tensor_scalar(den[:], den[:], -1.0, 2.0,
                                            ALU.mult, ALU.add)
                    rec2 = acts.tile([1, Lq], F16, name=f"rec2_{tag}_{h}",
                                     tag=f"rec2_{h}")
                    nc.vector.tensor_tensor(rec2[:], rec[:], den[:], ALU.mult)
                    rec2s.append(rec2)
                pos = []
                for h in range(HL):
                    for c in range(2):
                        po = pp(f"ps_o_{tag}_{h}_{c}", Lq)
                        for j in range(JT):
                            pj = min(128, Lkv - 128 * j)
                            nc.tensor.matmul(
                                po,
                                vT[j][0:pj, 256 * h + 128 * c:
                                      256 * h + 128 * (c + 1)],
                                exps_h[h][j][0:pj, :],
                                start=(j == 0), stop=(j == JT - 1))
                        pos.append(po)
                oT = []
                for h in range(HL):
                    rrep_p = pp(f"ps_rr_{tag}_{h}", Lq)
                    nc.tensor.matmul(rrep_p, ones_row[:], rec2s[h][:],
                                     start=True, stop=True)
                    rrep = acts.tile([128, Lq], F32, name=f"rr_{tag}_{h}",
                                     tag=f"rrep_{h}")
                    nc.scalar.copy(rrep[:], rrep_p)
                    for c in range(2):
                        o = acts.tile([128, Lq], F16,
                                      name=f"oT_{tag}_{h}_{c}",
                                      tag=f"oT_{2 * h + c}")
                        nc.vector.tensor_tensor(o[:], pos[2 * h + c],
                                                rrep[:], ALU.mult)
                        oT.append(o)
                return oT

            def out_proj_to_dram(tag, oT, w_dram, ar_in_halves,
                                 ar_out_halves, Lq):
                """4 quarter chunks; psum group of 8 m-tiles per quarter;
                AllReduce fired per half so the wire overlaps the rest."""
                for q in range(4):
                    wt = wchunk(f"wo_{tag}_{q}", 4 * 1024)
                    nc.sync.dma_start(
                        wt[:], w_dram.ap()[:, 4096 * q:4096 * (q + 1)])
                    ps = [pp(f"ps_op_{tag}_{q}_{mm}", Lq) for mm in range(8)]
                    for k in range(4):
                        for mm in range(8):
                            nc.tensor.matmul(
                                ps[mm],
                                wt[:, 1024 * k + 128 * mm:
                                   1024 * k + 128 * (mm + 1)],
                                oT[k][:],
                                start=(k == 0), stop=(k == 3))
                    h = q // 2
                    for sub in range(2):
                        st = acts.tile([128, 4 * Lq], F16,
                                       name=f"st_{tag}_{q}_{sub}",
                                       tag="stage", bufs=3,
                                       padded_shape=[128, 4 * 512])
                        for mi in range(4):
                            mm = 4 * sub + mi
                            nc.vector.tensor_copy(
                                st[:, mi * Lq:(mi + 1) * Lq], ps[mm])
                        off = ((8 * (q % 2)) + 4 * sub) * Lq
                        nc.sync.dma_start(
                            ar_in_halves[h][:, off:off + 4 * Lq], st[:])
                    if q == 1 or q == 3:
                        nc.gpsimd.collective_compute(
                            "AllReduce", ALU.add, replica_groups=replica,
                            ins=[ar_in_halves[h].opt()],
                            outs=[ar_out_halves[h].opt()])

            def ln_apply(x_ts, L, Asb, Bsb):
                for k in range(ET):
                    nc.vector.tensor_tensor(xs(x_ts, L, k), xs(x_ts, L, k),
                                            Asb[:], ALU.mult)
                    nc.vector.tensor_tensor(xs(x_ts, L, k), xs(x_ts, L, k),
                                            Bsb[:], ALU.add)

            def residual_ln2(tag, ar_out_halves, x_ts, L, dump=None):
                s1p = pstat(f"ps_s1_{tag}", L)
                s2p = pstat(f"ps_s2_{tag}", L)
                CG = 4  # k-blocks per arb read chunk
                for g in range(ET // CG):
                    half, hoff = g // 4, (g % 4) * CG * L
                    b = acts.tile([128, CG * L], F16, name=f"arb_{tag}_{g}",
                                  tag="arb", bufs=3,
                                  padded_shape=[128, CG * 512])
                    nc.sync.dma_start(
                        b[:], ar_out_halves[half][:, hoff:hoff + CG * L])
                    gg, off = (CG * g) // KG, ((CG * g) % KG) * L
                    nc.vector.tensor_tensor(
                        x_ts[gg][:, off:off + CG * L], b[:],
                        x_ts[gg][:, off:off + CG * L], ALU.add)
                    for kk in range(CG):
                        k = CG * g + kk
                        nc.tensor.matmul(s1p, ones_col[:], xs(x_ts, L, k),
                                         start=(k == 0), stop=(k == ET - 1))
                    for kk in range(CG):
                        k = CG * g + kk
                        sq = acts.tile([128, L], F16, name=f"sq_{tag}_{k}",
                                       tag="sqt", bufs=3,
                                       padded_shape=[128, 512])
                        nc.scalar.square(sq[:], xs(x_ts, L, k))
                        nc.tensor.matmul(s2p, ones_col[:], sq[:],
                                         start=(k == 0), stop=(k == ET - 1))
                mean = acts.tile([1, L], F32, name=f"mean_{tag}", tag="lmean")
                var = acts.tile([1, L], F32, name=f"var_{tag}", tag="lvar")
                tmpa = acts.tile([1, L], F32, name=f"tmpa_{tag}", tag="ltmp")
                r0 = acts.tile([1, L], F32, name=f"r0_{tag}", tag="lr0")
                nc.scalar.mul(mean[:], s1p, 1.0 / E)
                nc.scalar.mul(var[:], s2p, 1.0 / E)
                nc.scalar.square(tmpa[:], mean[:])
                nc.vector.tensor_sub(var[:], var[:], tmpa[:])
                nc.vector.tensor_scalar_add(var[:], var[:], 1e-5)
                nc.scalar.sqrt(tmpa[:], var[:])
                nc.vector.reciprocal(r0[:], tmpa[:])
                nc.vector.tensor_tensor(tmpa[:], r0[:], r0[:], ALU.mult)
                nc.vector.tensor_tensor(tmpa[:], tmpa[:], var[:], ALU.mult)
                nc.vector.tensor_scalar(tmpa[:], tmpa[:], -0.5, 1.5, ALU.mult,
                                        ALU.add)
                rstd = acts.tile([1, L], F16, name=f"rstd_{tag}", tag="rstd")
                nmr = acts.tile([1, L], F16, name=f"nmr_{tag}", tag="nmr")
                nc.vector.tensor_tensor(rstd[:], r0[:], tmpa[:], ALU.mult)
                nc.vector.scalar_tensor_tensor(nmr[:], mean[:], -1.0, rstd[:],
                                               ALU.mult, ALU.mult)
                Apsum = pp(f"ps_A_{tag}", L)
                nc.tensor.matmul(Apsum, ones_row[:], rstd[:], start=True,
                                 stop=True)
                Bpsum = pp(f"ps_B_{tag}", L)
                nc.tensor.matmul(Bpsum, ones_row[:], nmr[:], start=True,
                                 stop=True)
                Asb = acts.tile([128, L], F16, name=f"A_{tag}", tag="Asb")
                nc.scalar.copy(Asb[:], Apsum)
                Bsb = acts.tile([128, L], F16, name=f"B_{tag}", tag="Bsb")
                nc.scalar.copy(Bsb[:], Bpsum)
                ln_apply(x_ts, L, Asb, Bsb)
                if dump is not None:
                    for g in range(ET // KG):
                        nc.sync.dma_start(
                            dump.ap()[:, KG * L * g:KG * L * (g + 1)],
                            x_ts[g][:])

            # ================= program =================
            rem_ts = load_x("remx", remp_d, nrem, NG_R)
            cat_ts = load_x("catx", catp_d, ncat, NG_C)

            # ---- MHA2 (rem self-attention) ----
            q2 = proj_fm("q2", wd["q2"], rem_ts, nrem, "q")
            k2 = proj_fm("k2", wd["k2"], rem_ts, nrem, "k")
            v2 = proj_tm("v2", wd["v2"], rem_ts, nrem, JR)
            o2 = attention("a2", q2, k2, v2, nrem, nrem, JR)
            arin2 = [dram.tile([128, 16 * nrem], F16, name=f"arin2{h}",
                               tag=f"arin2{h}") for h in range(2)]
            arout2 = [dram.tile([128, 16 * nrem], F16, name=f"arout2{h}",
                                tag=f"arout2{h}", addr_space="Shared")
                      for h in range(2)]
            out_proj_to_dram("op2", o2, wd["o2"], arin2, arout2, nrem)

            # ---- MHA1 (cat self-attention), overlaps AR2 ----
            q1 = proj_fm("q1", wd["q1"], cat_ts, ncat, "q")
            k1 = proj_fm("k1", wd["k1"], cat_ts, ncat, "k")
            v1 = proj_tm("v1", wd["v1"], cat_ts, ncat, JC)
            o1 = attention("a1", q1, k1, v1, ncat, ncat, JC)
            arin1 = [dram.tile([128, 16 * ncat], F16, name=f"arin1{h}",
                               tag=f"arin1{h}") for h in range(2)]
            arout1 = [dram.tile([128, 16 * ncat], F16, name=f"arout1{h}",
                                tag=f"arout1{h}", addr_space="Shared")
                      for h in range(2)]
            out_proj_to_dram("op1", o1, wd["o1"], arin1, arout1, ncat)

            # ---- LN stages: r = LN(AR2 + rem); x1 = LN(AR1 + cat) ----
            residual_ln2("r", arout2, rem_ts, nrem, dump=dbg.get("dbg_r"))
            residual_ln2("x1", arout1, cat_ts, ncat, dump=dbg.get("dbg_x1"))

            # ---- MHAc (q from r, kv from x1) ----
            qc = proj_fm("qc", wd["qc"], rem_ts, nrem, "q")
            kc = proj_fm("kc", wd["kc"], cat_ts, ncat, "k")
            vc = proj_tm("vc", wd["vc"], cat_ts, ncat, JC)
            oc = attention("ac", qc, kc, vc, nrem, ncat, JC)
            arinc = [dram.tile([128, 16 * nrem], F16, name=f"arinc{h}",
                               tag=f"arinc{h}") for h in range(2)]
            aroutc = [dram.tile([128, 16 * nrem], F16, name=f"aroutc{h}",
                                tag=f"aroutc{h}", addr_space="Shared")
                      for h in range(2)]
            out_proj_to_dram("opc", oc, wd["oc"], arinc, aroutc, nrem)
            residual_ln2("x2", aroutc, rem_ts, nrem, dump=dbg.get("dbg_x2"))

            # ---- linear logit stat: Ws . x2 ----
            wxp = pstat("ps_wx2", nrem)
            for k in range(ET):
                nc.tensor.matmul(wxp, ws_sb[:, k:k + 1],
                                 xs(rem_ts, nrem, k),
                                 start=(k == 0), stop=(k == ET - 1))
            wx2 = acts.tile([1, nrem], F32, name="wx2", tag="wx2")
            nc.vector.tensor_copy(wx2[:], wxp)

            # ---- FFN f1: hT = gelu(Wf1_shard @ x2) ----
            f1chunks = []
            for g in range(ET // KG):
                wt = wchunk(f"w_f1_{g}", KG * FLOC)
                nc.sync.dma_start(
                    wt[:], wd["f1"].ap()[:, KG * FLOC * g:KG * FLOC * (g + 1)])
                f1chunks.append(wt)
            ps_f1 = [pp(f"ps_f1_{m}", nrem) for m in range(8)]
            for k in range(ET):
                g, kk = k // KG, k % KG
                for m in range(8):
                    nc.tensor.matmul(
                        ps_f1[m],
                        f1chunks[g][:, kk * FLOC + 128 * m:
                                    kk * FLOC + 128 * (m + 1)],
                        xs(rem_ts, nrem, k),
                        start=(k == 0), stop=(k == ET - 1))
            hT = []
            for m in range(8):
                h = acts.tile([128, nrem], F16, name=f"hT_{m}", tag=f"hT_{m}")
                nc.scalar.activation(h[:], ps_f1[m], AF.Gelu)
                hT.append(h)

            # ---- linear logit stats from hT: s1 = c2.g ; ws += w2s.g/256 ----
            c2p = pstat("ps_c2", nrem)
            w2p = pstat("ps_w2s", nrem)
            for m in range(8):
                nc.tensor.matmul(c2p, c2w_sb[:, m:m + 1], hT[m][:],
                                 start=(m == 0), stop=(m == 7))
            for m in range(8):
                nc.tensor.matmul(w2p, c2w_sb[:, 8 + m:9 + m], hT[m][:],
                                 start=(m == 0), stop=(m == 7))
            # stat staging rows padded to 512 so each maps onto a [128, 4]
            # block of the tiny-AR tensor (tail math then runs 128-wide)
            s1part = acts.tile([1, 512], F32, name="s1part", tag="s1part")
            wspart = acts.tile([1, 512], F32, name="wspart", tag="wspart")
            nc.vector.memset(s1part[:], 1.0)
            nc.vector.memset(wspart[:], 0.0)
            nc.vector.tensor_copy(s1part[:, 0:nrem], c2p)
            nc.vector.tensor_scalar(wx2[:], wx2[:], 1.0 / NCORES, 0.0,
                                    ALU.mult, ALU.add)
            nc.vector.scalar_tensor_tensor(wspart[:, 0:nrem], w2p,
                                           1.0 / 256.0, wx2[:],
                                           ALU.mult, ALU.add)
            arin6 = dram.tile([128, 16], F32, name="arin6", tag="arin6")
            arout6 = dram.tile([128, 16], F32, name="arout6", tag="arout6",
                               addr_space="Shared")
            nc.sync.dma_start(arin6[:, 0:4], s1part[:])
            nc.sync.dma_start(arin6[:, 4:8], wspart[:])
            nc.sync.dma_start(arin6[:, 12:16], s1part[:])

            # ---- FFN f2: partial = x2/8 + Wf2_shard^T hT; RS per half ----
            arin4 = [dram.tile([128, 16 * nrem], F16, name=f"arin4{h}",
                               tag=f"arin4{h}") for h in range(2)]
            rs4 = [dram.tile([16, 16 * nrem], F16, name=f"rs4{h}",
                             tag=f"rs4{h}") for h in range(2)]
            for q in range(4):
                wt = wchunk(f"w_f2_{q}", 8 * 1024)
                nc.sync.dma_start(
                    wt[:], wd["f2"].ap()[:, 8192 * q:8192 * (q + 1)])
                ps = [pp(f"ps_f2_{q}_{mm}", nrem) for mm in range(8)]
                for k in range(8):
                    for mm in range(8):
                        nc.tensor.matmul(
                            ps[mm],
                            wt[:, 1024 * k + 128 * mm:
                               1024 * k + 128 * (mm + 1)],
                            hT[k][:],
                            start=(k == 0), stop=(k == 7))
                for sub in range(2):
                    st = acts.tile([128, 4 * nrem], F16,
                                   name=f"st_f2_{q}_{sub}",
                                   tag="stage", bufs=3,
                                   padded_shape=[128, 4 * 512])
                    for mi in range(4):
                        mm = 4 * sub + mi
                        m = 8 * q + mm
                        nc.vector.scalar_tensor_tensor(
                            st[:, mi * nrem:(mi + 1) * nrem],
                            xs(rem_ts, nrem, m), 1.0 / NCORES, ps[mm],
                            ALU.mult, ALU.add)
                    off = ((8 * (q % 2)) + 4 * sub) * nrem
                    nc.sync.dma_start(
                        arin4[q // 2][:, off:off + 4 * nrem], st[:])
                if q == 1 or q == 3:
                    nc.gpsimd.collective_compute(
                        "ReduceScatter", ALU.add, replica_groups=replica,
                        ins=[arin4[q // 2].opt()],
                        outs=[rs4[q // 2].opt()])

            # ---- s2 from the scattered summed features ----
            NCH = 8
            CW = ET * nrem // NCH  # columns per rs4 read chunk
            NSUB = CW // nrem  # k-subblocks per chunk
            s2p = pstat("ps_rs2", nrem)
            for cch in range(NCH):
                hh, hcol = cch // 4, (cch % 4) * CW
                bt = acts.tile([16, CW], F16, name=f"rsb_{cch}", tag="rsb",
                               bufs=2, padded_shape=[16, 4 * 512])
                nc.sync.dma_start(bt[:], rs4[hh][:, hcol:hcol + CW])
                sq = acts.tile([16, CW], F16, name=f"rssq_{cch}", tag="rssq",
                               bufs=2, padded_shape=[16, 4 * 512])
                nc.scalar.square(sq[:], bt[:])
                for s in range(NSUB):
                    k = cch * NSUB + s
                    nc.tensor.matmul(s2p, ones_col[0:16, :],
                                     sq[:, s * nrem:(s + 1) * nrem],
                                     start=(k == 0), stop=(k == ET - 1))
            s2part = acts.tile([1, 512], F32, name="s2part", tag="s2part")
            nc.vector.memset(s2part[:], 1.0)
            nc.vector.tensor_copy(s2part[:, 0:nrem], s2p)
            nc.sync.dma_start(arin6[:, 8:12], s2part[:])
            nc.gpsimd.collective_compute(
                "AllReduce", ALU.add, replica_groups=replica,
                ins=[arin6.opt()], outs=[arout6.opt()])

            # ---- final logit: affine-LN identity, 128-wide blocked ----
            gsb = acts.tile([128, 12], F32, name="gsb", tag="gsb")
            nc.sync.dma_start(gsb[:], arout6[:, 0:12])
            g1, g2, g3 = gsb[:, 0:4], gsb[:, 4:8], gsb[:, 8:12]
            mean = acts.tile([128, 4], F32, name="mean_l", tag="lmean")
            var = acts.tile([128, 4], F32, name="var_l", tag="lvar")
            tmpa = acts.tile([128, 4], F32, name="tmpa_l", tag="ltmp")
            r0 = acts.tile([128, 4], F32, name="r0_l", tag="lr0")
            nc.scalar.mul(mean[:], g1, 1.0 / E)
            nc.scalar.mul(var[:], g3, 1.0 / E)
            nc.scalar.square(tmpa[:], mean[:])
            nc.vector.tensor_sub(var[:], var[:], tmpa[:])
            nc.vector.tensor_scalar_add(var[:], var[:], 1e-5)
            nc.scalar.sqrt(tmpa[:], var[:])
            nc.vector.reciprocal(r0[:], tmpa[:])
            nc.vector.tensor_tensor(tmpa[:], r0[:], r0[:], ALU.mult)
            nc.vector.tensor_tensor(tmpa[:], tmpa[:], var[:], ALU.mult)
            nc.vector.tensor_scalar(tmpa[:], tmpa[:], -0.5, 1.5,
                                    ALU.mult, ALU.add)
            rstd = acts.tile([128, 4], F32, name="rstd_l", tag="rstd_l")
            nc.vector.tensor_tensor(rstd[:], r0[:], tmpa[:], ALU.mult)
            nmr = acts.tile([128, 4], F32, name="nmr_l", tag="nmr_l")
            nc.vector.scalar_tensor_tensor(nmr[:], mean[:], -1.0,
                                           rstd[:], ALU.mult, ALU.mult)
            wdot = acts.tile([128, 4], F32, name="wdot", tag="wdot")
            nc.vector.tensor_tensor(wdot[:], rstd[:], g2, ALU.mult)
            lsb = acts.tile([128, 4], F32, name="lsb", tag="lsb")
            nc.vector.scalar_tensor_tensor(lsb[:], nmr[:],
                                           consts[:, 0:1], wdot[:],
                                           ALU.mult, ALU.add)
            nc.sync.dma_start(logits_d.ap(), lsb[:])

    nc.compile()
    return nc


# ----------------------------------------------------------------------------
# host orchestration
# ----------------------------------------------------------------------------

def _packx(XT):
    """[E, L] fp32 -> [128, ET*L] fp16 feature-block pack."""
    L = XT.shape[1]
    return np.ascontiguousarray(
        XT.reshape(ET, 128, L).transpose(1, 0, 2).reshape(128, ET * L)
        .astype(np.float16))


def _prep_in_maps(vision_feature, text_embed, sel_idx, rem_idx,
                  Wqkv1, Wo1, Wqkv2, Wo2, Wqkvc, Woc, Wf1, Wf2, Ws):
    f16 = np.float16
    sel = vision_feature[sel_idx]
    rem = vision_feature[rem_idx]
    cat = np.concatenate([sel, text_embed], axis=0)

    remp = _packx(np.ascontiguousarray(rem.T))
    catp = _packx(np.ascontiguousarray(cat.T))
    ws_pack = np.ascontiguousarray(Ws[0].reshape(ET, 128).T.astype(f16))
    consts = np.broadcast_to(
        np.array([[np.float64(Ws.astype(np.float64).sum()), 0.0]],
                 np.float32), (128, 2)).copy()

    in_maps = []
    for c in range(NCORES):
        hs = slice(DLOC * c, DLOC * (c + 1))
        fs = slice(FLOC * c, FLOC * (c + 1))
        m = {"remp": remp, "catp": catp, "wsp": ws_pack, "consts": consts}
        for l, Wqkv, Wo in (("1", Wqkv1, Wo1), ("2", Wqkv2, Wo2),
                            ("c", Wqkvc, Woc)):
            Wq, Wk, Wv = Wqkv[:E], Wqkv[E:2 * E], Wqkv[2 * E:]
            for nm, W in (("q", Wq), ("k", Wk), ("v", Wv)):
                A = W[hs].T  # [E, DLOC]
                m[f"w{nm}{l}"] = np.ascontiguousarray(
                    A.reshape(ET, 128, DLOC).transpose(1, 0, 2)
                    .reshape(128, ET * DLOC).astype(f16))
            WoT = Wo[:, hs].T  # [DLOC, E]
            m[f"wo{l}"] = np.ascontiguousarray(
                WoT.reshape(4, 128, 4, 1024).transpose(1, 2, 0, 3)
                .reshape(128, 4 * E).astype(f16))
        A = Wf1[fs].T  # [E, FLOC]
        m["wf1"] = np.ascontiguousarray(
            A.reshape(ET, 128, FLOC).transpose(1, 0, 2)
            .reshape(128, ET * FLOC).astype(f16))
        W2T = Wf2[:, fs].T  # [FLOC, E]
        m["wf2"] = np.ascontiguousarray(
            W2T.reshape(8, 128, 4, 1024).transpose(1, 2, 0, 3)
            .reshape(128, 8 * E).astype(f16))
        c2 = Wf2[:, fs].astype(np.float64).sum(axis=0)  # [FLOC]
        w2s = 256.0 * (Ws[0].astype(np.float64) @ Wf2[:, fs].astype(np.float64))
        c2w = np.concatenate([c2.reshape(8, 128).T, w2s.reshape(8, 128).T],
                             axis=1)  # [128, 16]
        m["c2w"] = np.ascontiguousarray(c2w.astype(f16))
        in_maps.append(m)
    return in_maps


def run_device(in_maps, ncat_real, nrem_real, dumps=False, trace=False):
    from concourse.bass_utils import run_bass_kernel_spmd

    key = (ncat_real, nrem_real, dumps)
    if key not in _CACHE:
        _CACHE[key] = _build_device(ncat_real, nrem_real, dumps=dumps)
    nc = _CACHE[key]
    return run_bass_kernel_spmd(nc, in_maps, list(range(NCORES)), trace=trace)


def _kernel_impl(inputs, debug=False, trace=False):
    vision_feature = np.asarray(inputs["vision_feature"], np.float32)
    text_embed = np.asarray(inputs["text_embed"], np.float32)
    attention_mask = np.asarray(inputs["attention_mask"])

    biases_zero = all(
        not np.any(np.asarray(inputs[b]))
        for b in ("bqkv1", "bo1", "bqkv2", "bo2", "bqkvc", "boc",
                  "bf1", "bf2", "bs"))
    if (not bool(attention_mask.all())) or (not biases_zero):
        return (_reference_np(**{k: np.asarray(v) for k, v in inputs.items()}),
                None)

    t, sel_idx, rem_idx = _score_partition(vision_feature, text_embed,
                                           attention_mask)
    ncat_real = t + text_embed.shape[0]
    nrem_real = vision_feature.shape[0] - t
    kk = int(t * EXPAND)

    in_maps = _prep_in_maps(
        vision_feature, text_embed, sel_idx, rem_idx,
        np.asarray(inputs["Wqkv1"], np.float32),
        np.asarray(inputs["Wo1"], np.float32),
        np.asarray(inputs["Wqkv2"], np.float32),
        np.asarray(inputs["Wo2"], np.float32),
        np.asarray(inputs["Wqkvc"], np.float32),
        np.asarray(inputs["Woc"], np.float32),
        np.asarray(inputs["Wf1"], np.float32),
        np.asarray(inputs["Wf2"], np.float32),
        np.asarray(inputs["Ws"], np.float32))
    res = run_device(in_maps, ncat_real, nrem_real, dumps=debug, trace=trace)
    logits = res.results[0]["logits"][0, :nrem_real]
    es = (1.0 / (1.0 + np.exp(-logits.astype(np.float32))))
    ei = np.argsort(-es, kind="stable")[:kk]
    final = np.sort(np.concatenate([sel_idx, rem_idx[ei]]))
    return vision_feature[final], res


def kernel(**inputs):
    out, _ = _kernel_impl(inputs)
    return out
